# revision 1
# baseline (speedup 1.0000x reference)
"""Trainium2 Bass kernel for a local-attention block (MQA, RoPE, causal mask).

Reference computation (B=2, T=2048, WIDTH=2560, 10 q-heads, 1 kv-head,
head_dim=256, window=2048 => mask reduces to causal & same-segment):

    q = x @ wq.T ; k = x @ wk.T ; v = x @ wv.T
    q, k = rope(q), rope(k)
    probs = softmax(q k^T / 16 + mask)
    out = (probs @ v) @ w_final.T + b_final

Sharding: 8 cores = 2 batches x 4 query-blocks of 512 tokens. The single
shared KV head is computed per-core (replicated within a batch). Per-core
host-side token ROTATION by the q-block start makes the device program
identical on every core (SPMD): the core's queries always sit in columns
0:512 of its rotated token axis, and causality is carried by a per-core
0/1 mask input.

Device layouts are "feature on partitions, tokens on free dim" so that
every matmul contraction maps to the 128-partition axis with no on-device
transposes (except 128x128 PE transposes of the attention output).
"""

import sys

import numpy as np

for _p in ("/opt/trn_rl_repo", "/root/.axon_site/_ro/trn_rl_repo"):
    if _p not in sys.path:
        sys.path.insert(0, _p)

import ml_dtypes

BF16 = ml_dtypes.bfloat16

B, T, WIDTH = 2, 2048, 2560
NUM_HEADS, HEAD_DIM = 10, 256
WINDOW = 2048
MAX_WAVELENGTH = 10000.0
QBLK = 512              # query tokens per core
NW = WIDTH // 128       # 20 width stripes
NTT = T // 128          # 16 token tiles
NQS = QBLK // 128       # 4 query sub-tiles
VROW = HEAD_DIM + 1     # v columns + ones column (denominator trick)

_NC_CACHE = {}


def _build_nc():
    """Build the (single, SPMD-uniform) Bass/Tile program."""
    import concourse.bass as bass  # noqa: F401
    import concourse.mybir as mybir
    import concourse.tile as tile
    from concourse import bacc
    from concourse.masks import make_identity

    fp32 = mybir.dt.float32
    bf16 = mybir.dt.bfloat16
    Exp = mybir.ActivationFunctionType.Exp

    nc = bacc.Bacc("TRN2", target_bir_lowering=False, debug=False)

    # ---- DRAM I/O ----
    xT = nc.dram_tensor("xT", [NW, 128, T], bf16, kind="ExternalInput")
    wq = nc.dram_tensor("wq", [NW, 128, WIDTH], bf16, kind="ExternalInput")
    wk = nc.dram_tensor("wk", [NW, 128, HEAD_DIM], bf16, kind="ExternalInput")
    wv = nc.dram_tensor("wv", [NW, 128, HEAD_DIM], bf16, kind="ExternalInput")
    wf = nc.dram_tensor("wf", [NW, 128, WIDTH], bf16, kind="ExternalInput")
    csk = nc.dram_tensor("csk", [64, T], fp32, kind="ExternalInput")
    snk = nc.dram_tensor("snk", [64, T], fp32, kind="ExternalInput")
    csq = nc.dram_tensor("csq", [64, QBLK], fp32, kind="ExternalInput")
    snq = nc.dram_tensor("snq", [64, QBLK], fp32, kind="ExternalInput")
    msk = nc.dram_tensor("msk", [NTT, 128, QBLK], bf16, kind="ExternalInput")
    bia = nc.dram_tensor("bia", [128, NW], fp32, kind="ExternalInput")
    out = nc.dram_tensor("out", [NW, 128, QBLK], fp32, kind="ExternalOutput")

    with tile.TileContext(nc) as tc:
        with (
            tc.tile_pool(name="res", bufs=1) as res,
            tc.tile_pool(name="bigp", bufs=1) as bigp,
            tc.tile_pool(name="wstr", bufs=2) as wstr,
            tc.tile_pool(name="ptp", bufs=4) as ptp,
            tc.tile_pool(name="enp", bufs=4) as enp,
            tc.tile_pool(name="tmp", bufs=4) as tmpp,
            tc.tile_pool(name="rcp", bufs=4) as rcpp,
            tc.tile_pool(name="outp", bufs=3) as outp,
            tc.tile_pool(name="pp", bufs=2, space="PSUM") as pp,
            tc.tile_pool(name="stp", bufs=2, space="PSUM") as stp,
            tc.tile_pool(name="op", bufs=4, space="PSUM") as op,
        ):
            # ---- resident SBUF tiles ----
            qtr = res.tile([128, NW * QBLK], bf16, tag="qtr")    # rope'd Q^T stripes
            ktr0 = res.tile([128, T], bf16, tag="ktr0")          # rope'd K^T hd 0:128
            ktr1 = res.tile([128, T], bf16, tag="ktr1")          # K^T hd 128:256
            vsb = res.tile([128, NTT * VROW], bf16, tag="vsb")   # V tiles + ones col
            wkr = res.tile([128, NW * HEAD_DIM], bf16, tag="wkr")
            wvr = res.tile([128, NW * HEAD_DIM], bf16, tag="wvr")
            csk_s = res.tile([64, T], fp32, tag="csk")
            snk_s = res.tile([64, T], fp32, tag="snk")
            csq_s = res.tile([64, QBLK], fp32, tag="csq")
            snq_s = res.tile([64, QBLK], fp32, tag="snq")
            bia_s = res.tile([128, NW], fp32, tag="bia")
            ident = res.tile([128, 128], bf16, tag="ident")

            make_identity(nc, ident[:])

            # xbig: 20 width-stripes of x^T, freed (slot reused) after V phase
            xbig = bigp.tile([128, NW * T], bf16, tag="big")

            for k in range(NW):
                nc.sync.dma_start(out=xbig[:, k * T:(k + 1) * T], in_=xT[k])
            nc.sync.dma_start(
                out=wkr[:].rearrange("p (n m) -> p n m", n=NW),
                in_=wk[:].rearrange("n p m -> p n m"),
            )
            nc.sync.dma_start(
                out=wvr[:].rearrange("p (n m) -> p n m", n=NW),
                in_=wv[:].rearrange("n p m -> p n m"),
            )
            nc.sync.dma_start(out=csk_s[:], in_=csk[:])
            nc.sync.dma_start(out=snk_s[:], in_=snk[:])
            nc.sync.dma_start(out=csq_s[:], in_=csq[:])
            nc.sync.dma_start(out=snq_s[:], in_=snq[:])
            nc.sync.dma_start(out=bia_s[:], in_=bia[:])

            # ones columns of V (denominator of softmax via matmul)
            for t in range(NTT):
                nc.gpsimd.memset(vsb[:, t * VROW + HEAD_DIM: (t + 1) * VROW], 1.0)

            def rope_evict(ps, cs, sn, dst01, dst0, dst1):
                """dst[0:64] = ps0*cos - ps1*sin ; dst[64:128] = ps1*cos + ps0*sin.

                ps: [128, n] PSUM fp32; cs/sn: [64, n] SBUF fp32 tables;
                dst01/dst0/dst1: bf16 SBUF APs ([128,n],[0:64],[64:128])."""
                n = cs.shape[-1]
                t0 = tmpp.tile([64, QBLK], fp32, tag="t0", name="t0")
                t1 = tmpp.tile([64, QBLK], fp32, tag="t1", name="t1")
                nc.vector.tensor_mul(t0[:, :n], ps[0:64, :], cs)
                nc.vector.tensor_mul(t1[:, :n], ps[64:128, :], sn)
                nc.vector.tensor_sub(dst0, t0[:, :n], t1[:, :n])
                t2 = tmpp.tile([64, QBLK], fp32, tag="t0", name="t2")
                t3 = tmpp.tile([64, QBLK], fp32, tag="t1", name="t3")
                nc.vector.tensor_mul(t2[:, :n], ps[64:128, :], cs)
                nc.vector.tensor_mul(t3[:, :n], ps[0:64, :], sn)
                nc.vector.tensor_add(dst1, t2[:, :n], t3[:, :n])

            # ---- Q projection -> rope'd Q^T stripes [qdim, QBLK] ----
            # stripe m: qdim rows [128m, 128m+128) = head m//2, half m%2
            for m in range(NW):
                wq_m = wstr.tile([128, WIDTH], bf16, tag="w")
                nc.sync.dma_start(out=wq_m[:], in_=wq[m])
                ps = pp.tile([128, QBLK], fp32, tag="pp")
                for k in range(NW):
                    nc.tensor.matmul(
                        ps[:],
                        lhsT=wq_m[:, k * 128:(k + 1) * 128],
                        rhs=xbig[:, k * T: k * T + QBLK],
                        start=(k == 0),
                        stop=(k == NW - 1),
                    )
                dst = qtr[:, m * QBLK:(m + 1) * QBLK]
                if m % 2 == 0:  # rope half of the head dims
                    rope_evict(ps, csq_s[:], snq_s[:], dst,
                               qtr[0:64, m * QBLK:(m + 1) * QBLK],
                               qtr[64:128, m * QBLK:(m + 1) * QBLK])
                else:           # passthrough half
                    nc.scalar.copy(out=dst, in_=ps[:])

            # ---- K projection -> rope'd K^T [256, T] (2 stripes) ----
            for hh in range(2):
                for g in range(T // QBLK):
                    ps = pp.tile([128, QBLK], fp32, tag="pp")
                    for k in range(NW):
                        nc.tensor.matmul(
                            ps[:],
                            lhsT=wkr[:, k * HEAD_DIM + hh * 128:
                                     k * HEAD_DIM + hh * 128 + 128],
                            rhs=xbig[:, k * T + g * QBLK: k * T + (g + 1) * QBLK],
                            start=(k == 0),
                            stop=(k == NW - 1),
                        )
                    cols = slice(g * QBLK, (g + 1) * QBLK)
                    if hh == 0:
                        rope_evict(ps, csk_s[:, cols], snk_s[:, cols],
                                   ktr0[:, cols], ktr0[0:64, cols],
                                   ktr0[64:128, cols])
                    else:
                        nc.scalar.copy(out=ktr1[:, cols], in_=ps[:])

            # ---- V projection -> V tiles [tok, 256] + ones col ----
            for mt in range(NTT):
                ps = pp.tile([128, HEAD_DIM], fp32, tag="pp")
                for k in range(NW):
                    nc.tensor.matmul(
                        ps[:],
                        lhsT=xbig[:, k * T + mt * 128: k * T + (mt + 1) * 128],
                        rhs=wvr[:, k * HEAD_DIM:(k + 1) * HEAD_DIM],
                        start=(k == 0),
                        stop=(k == NW - 1),
                    )
                nc.scalar.copy(
                    out=vsb[:, mt * VROW: mt * VROW + HEAD_DIM], in_=ps[:])

            # masks + enc^T reuse xbig's SBUF slot once projections are done
            attnbuf = bigp.tile([128, NTT * QBLK + NW * QBLK], bf16, tag="big")
            masks = attnbuf[:, : NTT * QBLK]
            enct = attnbuf[:, NTT * QBLK:]
            for t in range(NTT):
                nc.sync.dma_start(
                    out=masks[:, t * QBLK:(t + 1) * QBLK], in_=msk[t])

            # ---- attention (S^T layout: k on partitions, q on free dim) ----
            for h in range(NUM_HEADS):
                o_tiles = [op.tile([128, VROW], fp32, tag="o", name=f"o{h}_{i}")
                           for i in range(NQS)]
                for t in range(NTT):
                    st = stp.tile([128, QBLK], fp32, tag="st")
                    nc.tensor.matmul(
                        st[:], lhsT=ktr0[:, t * 128:(t + 1) * 128],
                        rhs=qtr[:, (2 * h) * QBLK:(2 * h + 1) * QBLK],
                        start=True, stop=False)
                    nc.tensor.matmul(
                        st[:], lhsT=ktr1[:, t * 128:(t + 1) * 128],
                        rhs=qtr[:, (2 * h + 1) * QBLK:(2 * h + 2) * QBLK],
                        start=False, stop=True)
                    pt = ptp.tile([128, QBLK], bf16, tag="pt")
                    # p = exp(s / sqrt(head_dim)), masked entries -> 0
                    nc.scalar.activation(pt[:], st[:], Exp, scale=0.0625)
                    nc.vector.tensor_mul(
                        pt[:], pt[:], masks[:, t * QBLK:(t + 1) * QBLK])
                    for q4 in range(NQS):
                        nc.tensor.matmul(
                            o_tiles[q4][:],
                            lhsT=pt[:, q4 * 128:(q4 + 1) * 128],
                            rhs=vsb[:, t * VROW:(t + 1) * VROW],
                            start=(t == 0),
                            stop=(t == NTT - 1),
                        )
                for q4 in range(NQS):
                    r = rcpp.tile([128, 1], fp32, tag="r")
                    nc.vector.reciprocal(r[:], o_tiles[q4][:, HEAD_DIM:VROW])
                    en = enp.tile([128, HEAD_DIM], bf16, tag="en")
                    nc.vector.tensor_scalar_mul(
                        en[:], o_tiles[q4][:, 0:HEAD_DIM], r[:])
                    for hh in range(2):
                        tp = pp.tile([128, 128], bf16, tag="pp")
                        nc.tensor.matmul(
                            tp[:], lhsT=en[:, hh * 128:(hh + 1) * 128],
                            rhs=ident[:], is_transpose=True)
                        nc.vector.tensor_copy(
                            enct[:, (2 * h + hh) * QBLK + q4 * 128:
                                 (2 * h + hh) * QBLK + (q4 + 1) * 128],
                            tp[:])

            # ---- final projection: out^T = wf @ enc^T + bias ----
            for m in range(NW):
                wf_m = wstr.tile([128, WIDTH], bf16, tag="w")
                nc.sync.dma_start(out=wf_m[:], in_=wf[m])
                ps = pp.tile([128, QBLK], fp32, tag="pp")
                for k in range(NW):
                    nc.tensor.matmul(
                        ps[:],
                        lhsT=wf_m[:, k * 128:(k + 1) * 128],
                        rhs=enct[:, k * QBLK:(k + 1) * QBLK],
                        start=(k == 0),
                        stop=(k == NW - 1),
                    )
                osb = outp.tile([128, QBLK], fp32, tag="osb")
                nc.vector.tensor_scalar_add(osb[:], ps[:], bia_s[:, m:m + 1])
                nc.sync.dma_start(out=out[m], in_=osb[:])

    if not nc.is_finalized():
        nc.finalize()  # bacc register allocation — required before walrus compile
    return nc


def get_nc():
    if "nc" not in _NC_CACHE:
        _NC_CACHE["nc"] = _build_nc()
    return _NC_CACHE["nc"]


def _host_prepare(x, segment_pos, wq, wk, wv, w_final, b_final):
    """Build shared + per-core device input arrays."""
    x = np.asarray(x, dtype=np.float32)
    segment_pos = np.asarray(segment_pos)
    wq = np.asarray(wq, dtype=np.float32)
    wk = np.asarray(wk, dtype=np.float32)
    wv = np.asarray(wv, dtype=np.float32)
    w_final = np.asarray(w_final, dtype=np.float32)
    b_final = np.asarray(b_final, dtype=np.float32)

    def stripes_sq(w):  # [WIDTH, WIDTH] torch-Linear weight -> [20,128,WIDTH] w^T stripes
        wt = np.ascontiguousarray(w.T)
        return np.ascontiguousarray(
            wt.reshape(NW, 128, NW, 128).transpose(2, 1, 0, 3).reshape(NW, 128, WIDTH)
        ).astype(BF16)

    shared = {
        "wq": stripes_sq(wq),
        "wf": stripes_sq(w_final),
        "wk": np.ascontiguousarray(wk.T).reshape(NW, 128, HEAD_DIM).astype(BF16),
        "wv": np.ascontiguousarray(wv.T).reshape(NW, 128, HEAD_DIM).astype(BF16),
        "bia": np.ascontiguousarray(b_final.reshape(NW, 128).T).astype(np.float32),
    }

    inv_freq = (
        1.0 / MAX_WAVELENGTH ** (2.0 * np.arange(HEAD_DIM // 4, dtype=np.float32)
                                 / (HEAD_DIM // 2))
    ).astype(np.float32)

    in_maps = []
    for c in range(8):
        b = c // 4
        qs = QBLK * (c % 4)
        perm = (qs + np.arange(T)) % T  # rotated token order

        xrot = x[b][perm]  # [T, WIDTH]
        xTc = np.ascontiguousarray(xrot.T).astype(BF16).reshape(NW, 128, T)

        pos = segment_pos[b].astype(np.float32)
        ang = inv_freq[:, None] * pos[perm][None, :]  # [64, T]
        csk_ = np.cos(ang).astype(np.float32)
        snk_ = np.sin(ang).astype(np.float32)

        # allow[tq, tk] on original token ids (causal & window & same segment)
        seg = np.cumsum((segment_pos[b] == 0).astype(np.int64))
        tq = qs + np.arange(QBLK)
        tk = perm
        allow = (
            (tk[None, :] <= tq[:, None])
            & (tq[:, None] <= tk[None, :] + WINDOW)
            & (seg[tq][:, None] == seg[tk][None, :])
        )
        mask_kq = np.ascontiguousarray(allow.T).astype(BF16)  # [T(k rot), QBLK]

        in_maps.append(dict(
            shared,
            xT=xTc,
            csk=csk_,
            snk=snk_,
            csq=np.ascontiguousarray(csk_[:, :QBLK]),
            snq=np.ascontiguousarray(snk_[:, :QBLK]),
            msk=mask_kq.reshape(NTT, 128, QBLK),
        ))
    return in_maps


def _assemble(results):
    out = np.empty((B, T, WIDTH), dtype=np.float32)
    for c, res in enumerate(results):
        b, qs = c // 4, QBLK * (c % 4)
        o = np.asarray(res["out"], dtype=np.float32)  # [NW, 128, QBLK]
        out[b, qs:qs + QBLK, :] = o.transpose(2, 0, 1).reshape(QBLK, WIDTH)
    return out


def kernel(x, segment_pos, wq, wk, wv, w_final, b_final):
    from concourse.bass_utils import run_bass_kernel_spmd

    nc = get_nc()
    in_maps = _host_prepare(x, segment_pos, wq, wk, wv, w_final, b_final)
    res = run_bass_kernel_spmd(nc, in_maps, list(range(8)))
    return _assemble(res.results)



# revision 7
# speedup vs baseline: 1.4038x; 1.4038x over previous
"""Trainium2 Bass kernel for a local-attention block (MQA, RoPE, causal mask).

Reference computation (B=2, T=2048, WIDTH=2560, 10 q-heads, 1 kv-head,
head_dim=256, window=2048 => mask reduces to causal & same-segment):

    q = x @ wq.T ; k = x @ wk.T ; v = x @ wv.T
    q, k = rope(q), rope(k)
    probs = softmax(q k^T / 16 + mask)
    out = (probs @ v) @ w_final.T + b_final

Sharding: 8 cores = 2 batches x 4 ranks. Load-balanced causal split: rank r
owns the four 128-token query chunks {r, r+4, r+8, r+12}, placed in "slots"
ordered by decreasing causal coverage (16, 12, 8, 4 key tiles of 128). For
key tile t only the first needed(t) = 4 - t//4 slots are processed, so the
S/O matmuls use contiguous column prefixes of the slot-ordered Q buffer and
every core does identical work (SPMD) with no fully-masked tiles computed.

K/V projection is sharded: each core projects+ropes K/V for its own 512
tokens, then an AllGather over the 4 cores of a batch reconstructs the full
K^T / V in every core. The gather runs on the collective cores concurrently
with the Q projection.

Device layouts are "feature on partitions, tokens on free dim" so every
matmul contraction maps to the 128-partition axis with no on-device
transposes (except 128x128 PE transposes of the attention output).
"""

import sys

import numpy as np

for _p in ("/opt/trn_rl_repo", "/root/.axon_site/_ro/trn_rl_repo"):
    if _p not in sys.path:
        sys.path.insert(0, _p)

import ml_dtypes

BF16 = ml_dtypes.bfloat16

B, T, WIDTH = 2, 2048, 2560
NUM_HEADS, HEAD_DIM = 10, 256
WINDOW = 2048
MAX_WAVELENGTH = 10000.0
QBLK = 512              # query tokens per core (4 chunks of 128, slot order)
KVB = 512               # kv tokens projected per core
NW = WIDTH // 128       # 20 width stripes
NTT = T // 128          # 16 key token tiles
VROW = HEAD_DIM + 1     # v columns + ones column (denominator trick)
NSLOT = 4
NCOV = [16, 12, 8, 4]   # key-tile coverage per slot
# key tiles grouped into shared PSUM banks (equal needed() within a group)
TGROUPS = [[0], [1], [2], [3], [4], [5], [6], [7], [8, 9], [10, 11],
           [12, 13, 14, 15]]


def _needed(t):
    return 4 - t // 4


# mask/pt column offset of key tile t
MOFF = [0] * NTT
for _t in range(1, NTT):
    MOFF[_t] = MOFF[_t - 1] + 128 * _needed(_t - 1)
MCOLS = MOFF[-1] + 128 * _needed(NTT - 1)  # 5120

_NC_CACHE = {}


def _build_nc():
    """Build the (single, SPMD-uniform) Bass/Tile program."""
    import concourse.bass as bass  # noqa: F401
    import concourse.mybir as mybir
    import concourse.tile as tile
    from concourse import bacc
    from concourse.masks import make_identity

    fp32 = mybir.dt.float32
    bf16 = mybir.dt.bfloat16
    Exp = mybir.ActivationFunctionType.Exp
    Copy = mybir.ActivationFunctionType.Copy

    nc = bacc.Bacc("TRN2", target_bir_lowering=False, debug=False, num_devices=8)

    # ---- DRAM I/O ----
    xq = nc.dram_tensor("xq", [NW, 128, QBLK], bf16, kind="ExternalInput")
    xkv = nc.dram_tensor("xkv", [NW, 128, KVB], bf16, kind="ExternalInput")
    wq = nc.dram_tensor("wq", [NW, 128, WIDTH], bf16, kind="ExternalInput")
    wkr_d = nc.dram_tensor("wkr", [128, NW * HEAD_DIM], bf16, kind="ExternalInput")
    wvr_d = nc.dram_tensor("wvr", [128, NW * HEAD_DIM], bf16, kind="ExternalInput")
    wf = nc.dram_tensor("wf", [NW, 128, WIDTH], bf16, kind="ExternalInput")
    csk = nc.dram_tensor("csk", [64, KVB], fp32, kind="ExternalInput")
    snk = nc.dram_tensor("snk", [64, KVB], fp32, kind="ExternalInput")
    csq = nc.dram_tensor("csq", [64, QBLK], fp32, kind="ExternalInput")
    snq = nc.dram_tensor("snq", [64, QBLK], fp32, kind="ExternalInput")
    msk = nc.dram_tensor("msk", [128, MCOLS], bf16, kind="ExternalInput")
    bia = nc.dram_tensor("bia", [128, NW], fp32, kind="ExternalInput")
    out = nc.dram_tensor("out", [NW, 128, QBLK], fp32, kind="ExternalOutput")

    with tile.TileContext(nc) as tc:
        with (
            tc.tile_pool(name="res", bufs=1) as res,
            tc.tile_pool(name="wstr", bufs=2) as wstr,
            tc.tile_pool(name="ptp", bufs=4) as ptp,
            tc.tile_pool(name="enp", bufs=4) as enp,
            tc.tile_pool(name="tmp", bufs=4) as tmpp,
            tc.tile_pool(name="rcp", bufs=4) as rcpp,
            tc.tile_pool(name="outp", bufs=3) as outp,
            tc.tile_pool(name="dram", bufs=1, space="DRAM") as dram,
            tc.tile_pool(name="pp", bufs=2, space="PSUM") as pp,
            tc.tile_pool(name="stp", bufs=2, space="PSUM") as stp,
            tc.tile_pool(name="op", bufs=4, space="PSUM") as op,
        ):
            # ---- resident SBUF tiles ----
            xkvs = res.tile([128, NW * KVB], bf16, tag="xkvs")
            xqs = res.tile([128, NW * QBLK], bf16, tag="xqs")
            qtr = res.tile([128, NW * QBLK], bf16, tag="qtr")   # rope'd Q^T
            ktr0 = res.tile([128, T], bf16, tag="ktr0")         # rope'd K^T hd 0:128
            ktr1 = res.tile([128, T], bf16, tag="ktr1")         # K^T hd 128:256
            vsb = res.tile([128, NTT * VROW], bf16, tag="vsb")  # V tiles + ones col
            kvsh = res.tile([128, 2048], bf16, tag="kvsh")      # own K/V shard
            wkr = res.tile([128, NW * HEAD_DIM], bf16, tag="wkr")
            wvr = res.tile([128, NW * HEAD_DIM], bf16, tag="wvr")
            enct = res.tile([128, NW * QBLK], bf16, tag="enct")
            csk_s = res.tile([64, KVB], fp32, tag="csk")
            snk_s = res.tile([64, KVB], fp32, tag="snk")
            csq_s = res.tile([64, QBLK], fp32, tag="csq")
            snq_s = res.tile([64, QBLK], fp32, tag="snq")
            msk_s = res.tile([128, MCOLS], bf16, tag="msk")
            bia_s = res.tile([128, NW], fp32, tag="bia")
            ident = res.tile([128, 128], bf16, tag="ident")

            make_identity(nc, ident[:])

            kv_in = dram.tile([128, 2048], bf16, tag="kvi")
            kv_out = dram.tile([4, 128, 2048], bf16, tag="kvo")

            # ---- input DMAs (SP queue, in consumption order) ----
            nc.sync.dma_start(out=wkr[:], in_=wkr_d[:])
            nc.sync.dma_start(out=wvr[:], in_=wvr_d[:])
            nc.sync.dma_start(
                out=xkvs[:].rearrange("p (k c) -> p k c", k=NW),
                in_=xkv[:].rearrange("k p c -> p k c"))
            nc.sync.dma_start(out=csk_s[:], in_=csk[:])
            nc.sync.dma_start(out=snk_s[:], in_=snk[:])
            nc.sync.dma_start(
                out=xqs[:].rearrange("p (k c) -> p k c", k=NW),
                in_=xq[:].rearrange("k p c -> p k c"))
            nc.sync.dma_start(out=csq_s[:], in_=csq[:])
            nc.sync.dma_start(out=snq_s[:], in_=snq[:])

            # ones columns of V (denominator of softmax via matmul)
            nc.gpsimd.memset(
                vsb[:].rearrange("p (n v) -> p n v", n=NTT)[:, :, HEAD_DIM:VROW],
                1.0)

            def rope_evict(ps, cs, sn, dst0, dst1):
                """dst0 = ps0*cos - ps1*sin ; dst1 = ps1*cos + ps0*sin.

                ps: [128, n] PSUM fp32; cs/sn: [64, n] SBUF fp32 tables;
                dst0/dst1: bf16 SBUF APs [64, n] (head-dim halves)."""
                n = cs.shape[-1]
                t0 = tmpp.tile([64, QBLK], fp32, tag="t0", name="t0")
                t1 = tmpp.tile([64, QBLK], fp32, tag="t1", name="t1")
                nc.vector.tensor_mul(t0[:, :n], ps[0:64, :], cs)
                nc.vector.tensor_mul(t1[:, :n], ps[64:128, :], sn)
                nc.vector.tensor_sub(dst0, t0[:, :n], t1[:, :n])
                t2 = tmpp.tile([64, QBLK], fp32, tag="t0", name="t2")
                t3 = tmpp.tile([64, QBLK], fp32, tag="t1", name="t3")
                nc.vector.tensor_mul(t2[:, :n], ps[64:128, :], cs)
                nc.vector.tensor_mul(t3[:, :n], ps[0:64, :], sn)
                nc.vector.tensor_add(dst1, t2[:, :n], t3[:, :n])

            # ---- K/V shard projection (own 512 tokens) ----
            # kvsh cols: [0:512] rope'd K^T hd0:128, [512:1024] K^T hd128:256,
            # [1024:2048] V tiles (4 x [128tok, 256hd])
            psk0 = pp.tile([128, KVB], fp32, tag="pp", name="psk0")
            psk1 = pp.tile([128, KVB], fp32, tag="pp", name="psk1")
            psv = [op.tile([128, HEAD_DIM], fp32, tag="o", name=f"psv{mt}")
                   for mt in range(4)]
            for k in range(NW):
                nc.tensor.matmul(
                    psk0[:], lhsT=wkr[:, k * HEAD_DIM: k * HEAD_DIM + 128],
                    rhs=xkvs[:, k * KVB:(k + 1) * KVB],
                    start=(k == 0), stop=(k == NW - 1))
                nc.tensor.matmul(
                    psk1[:], lhsT=wkr[:, k * HEAD_DIM + 128:(k + 1) * HEAD_DIM],
                    rhs=xkvs[:, k * KVB:(k + 1) * KVB],
                    start=(k == 0), stop=(k == NW - 1))
                for mt in range(4):
                    nc.tensor.matmul(
                        psv[mt][:],
                        lhsT=xkvs[:, k * KVB + mt * 128: k * KVB + (mt + 1) * 128],
                        rhs=wvr[:, k * HEAD_DIM:(k + 1) * HEAD_DIM],
                        start=(k == 0), stop=(k == NW - 1))
            rope_evict(psk0, csk_s[:], snk_s[:],
                       kvsh[0:64, 0:KVB], kvsh[64:128, 0:KVB])
            nc.scalar.copy(out=kvsh[:, KVB:2 * KVB], in_=psk1[:])
            for mt in range(4):
                nc.scalar.copy(
                    out=kvsh[:, 1024 + mt * HEAD_DIM: 1024 + (mt + 1) * HEAD_DIM],
                    in_=psv[mt][:])

            # ---- K/V all-gather across the 4 cores of this batch ----
            nc.gpsimd.dma_start(out=kv_in[:], in_=kvsh[:])
            nc.gpsimd.collective_compute(
                "AllGather",
                mybir.AluOpType.bypass,
                replica_groups=[[0, 1, 2, 3], [4, 5, 6, 7]],
                ins=[kv_in.opt()],
                outs=[kv_out.opt()],
            )
            nc.gpsimd.dma_start(
                out=ktr0[:].rearrange("p (r c) -> p r c", r=4),
                in_=kv_out[:, :, 0:KVB].rearrange("r p c -> p r c"))
            nc.gpsimd.dma_start(
                out=ktr1[:].rearrange("p (r c) -> p r c", r=4),
                in_=kv_out[:, :, KVB:2 * KVB].rearrange("r p c -> p r c"))
            for r4 in range(4):
                nc.gpsimd.dma_start(
                    out=vsb[:].rearrange("p (n v) -> p n v", n=NTT)[
                        :, 4 * r4:4 * (r4 + 1), 0:HEAD_DIM],
                    in_=kv_out[r4, :, 1024:2048].rearrange(
                        "p (t v) -> p t v", t=4))

            # ---- Q projection -> rope'd Q^T stripes [qdim, QBLK] ----
            # stripe m: qdim rows [128m, 128m+128) = head m//2, half m%2
            for m in range(NW):
                wq_m = wstr.tile([128, WIDTH], bf16, tag="w")
                nc.sync.dma_start(out=wq_m[:], in_=wq[m])
                ps = pp.tile([128, QBLK], fp32, tag="pp")
                for k in range(NW):
                    nc.tensor.matmul(
                        ps[:],
                        lhsT=wq_m[:, k * 128:(k + 1) * 128],
                        rhs=xqs[:, k * QBLK:(k + 1) * QBLK],
                        start=(k == 0),
                        stop=(k == NW - 1),
                    )
                dst = qtr[:, m * QBLK:(m + 1) * QBLK]
                if m % 2 == 0:  # rope half of the head dims
                    rope_evict(ps, csq_s[:], snq_s[:],
                               qtr[0:64, m * QBLK:(m + 1) * QBLK],
                               qtr[64:128, m * QBLK:(m + 1) * QBLK])
                else:           # passthrough half
                    nc.scalar.copy(out=dst, in_=ps[:])

            # masks + bias arrive behind the wq stripes, before attention
            nc.sync.dma_start(out=msk_s[:], in_=msk[:])
            nc.sync.dma_start(out=bia_s[:], in_=bia[:])

            # ---- attention (S^T layout: k on partitions, q on free dim) ----
            def evict_slot(h, p, o_tile):
                r = rcpp.tile([128, 1], fp32, tag="r")
                nc.vector.reciprocal(r[:], o_tile[:, HEAD_DIM:VROW])
                en = enp.tile([128, HEAD_DIM], bf16, tag="en")
                nc.scalar.activation(en[:], o_tile[:, 0:HEAD_DIM], Copy,
                                     scale=r[:])
                for hh in range(2):
                    tp = pp.tile([128, 128], bf16, tag="pp")
                    nc.tensor.matmul(
                        tp[:], lhsT=en[:, hh * 128:(hh + 1) * 128],
                        rhs=ident[:], is_transpose=True)
                    nc.vector.tensor_copy(
                        enct[:, (2 * h + hh) * QBLK + p * 128:
                             (2 * h + hh) * QBLK + (p + 1) * 128],
                        tp[:])

            for h in range(NUM_HEADS):
                o_tiles = [op.tile([128, VROW], fp32, tag="o", name=f"o{h}_{p}")
                           for p in range(NSLOT)]
                for grp in TGROUPS:
                    nd = _needed(grp[0])
                    gw = 128 * nd * len(grp)   # group column width
                    st = stp.tile([128, QBLK], fp32, tag="st")
                    for j, t in enumerate(grp):
                        cols = slice(j * 128 * nd, (j + 1) * 128 * nd)
                        nc.tensor.matmul(
                            st[:, cols], lhsT=ktr0[:, t * 128:(t + 1) * 128],
                            rhs=qtr[:, (2 * h) * QBLK:
                                     (2 * h) * QBLK + 128 * nd],
                            start=True, stop=False)
                        nc.tensor.matmul(
                            st[:, cols], lhsT=ktr1[:, t * 128:(t + 1) * 128],
                            rhs=qtr[:, (2 * h + 1) * QBLK:
                                     (2 * h + 1) * QBLK + 128 * nd],
                            start=False, stop=True)
                    pt = ptp.tile([128, QBLK], bf16, tag="pt")
                    # p = exp(s / sqrt(head_dim)), masked entries -> 0
                    nc.scalar.activation(pt[:, :gw], st[:, :gw], Exp,
                                         scale=0.0625)
                    nc.vector.tensor_mul(
                        pt[:, :gw], pt[:, :gw],
                        msk_s[:, MOFF[grp[0]]:MOFF[grp[0]] + gw])
                    for j, t in enumerate(grp):
                        for p in reversed(range(nd)):
                            nc.tensor.matmul(
                                o_tiles[p][:],
                                lhsT=pt[:, j * 128 * nd + p * 128:
                                        j * 128 * nd + (p + 1) * 128],
                                rhs=vsb[:, t * VROW:(t + 1) * VROW],
                                start=(t == 0),
                                stop=(t == NCOV[p] - 1),
                            )
                    # evict any slot whose accumulation just completed
                    tlast = grp[-1]
                    for p in range(NSLOT):
                        if NCOV[p] - 1 == tlast:
                            evict_slot(h, p, o_tiles[p])

            # ---- final projection: out^T = wf @ enc^T + bias ----
            for m in range(NW):
                wf_m = wstr.tile([128, WIDTH], bf16, tag="w")
                nc.sync.dma_start(out=wf_m[:], in_=wf[m])
                ps = pp.tile([128, QBLK], fp32, tag="pp")
                for k in range(NW):
                    nc.tensor.matmul(
                        ps[:],
                        lhsT=wf_m[:, k * 128:(k + 1) * 128],
                        rhs=enct[:, k * QBLK:(k + 1) * QBLK],
                        start=(k == 0),
                        stop=(k == NW - 1),
                    )
                osb = outp.tile([128, QBLK], fp32, tag="osb")
                nc.vector.tensor_scalar_add(osb[:], ps[:], bia_s[:, m:m + 1])
                nc.sync.dma_start(out=out[m], in_=osb[:])

    if not nc.is_finalized():
        nc.finalize()  # bacc register allocation — required before walrus compile
    return nc


def get_nc():
    if "nc" not in _NC_CACHE:
        _NC_CACHE["nc"] = _build_nc()
    return _NC_CACHE["nc"]


def _chunk_of_slot(r, p):
    """Physical 128-token query chunk held by rank r's slot p."""
    return r + 12 - 4 * p


def _host_prepare(x, segment_pos, wq, wk, wv, w_final, b_final):
    """Build shared + per-core device input arrays."""
    x = np.asarray(x, dtype=np.float32)
    segment_pos = np.asarray(segment_pos)
    wq = np.asarray(wq, dtype=np.float32)
    wk = np.asarray(wk, dtype=np.float32)
    wv = np.asarray(wv, dtype=np.float32)
    w_final = np.asarray(w_final, dtype=np.float32)
    b_final = np.asarray(b_final, dtype=np.float32)

    def stripes_sq(w):  # [WIDTH, WIDTH] torch-Linear weight -> [20,128,WIDTH] w^T stripes
        wt = np.ascontiguousarray(w.T)
        return np.ascontiguousarray(
            wt.reshape(NW, 128, NW, 128).transpose(2, 1, 0, 3).reshape(NW, 128, WIDTH)
        ).astype(BF16)

    def packed_kv(w):  # [HEAD_DIM, WIDTH] -> [128, NW*HEAD_DIM] w^T stripes
        return np.ascontiguousarray(
            w.T.reshape(NW, 128, HEAD_DIM).transpose(1, 0, 2).reshape(
                128, NW * HEAD_DIM)
        ).astype(BF16)

    shared = {
        "wq": stripes_sq(wq),
        "wf": stripes_sq(w_final),
        "wkr": packed_kv(wk),
        "wvr": packed_kv(wv),
        "bia": np.ascontiguousarray(b_final.reshape(NW, 128).T).astype(np.float32),
    }

    inv_freq = (
        1.0 / MAX_WAVELENGTH ** (2.0 * np.arange(HEAD_DIM // 4, dtype=np.float32)
                                 / (HEAD_DIM // 2))
    ).astype(np.float32)

    def cossin(pos):
        ang = inv_freq[:, None] * pos[None, :].astype(np.float32)
        return (np.cos(ang).astype(np.float32), np.sin(ang).astype(np.float32))

    in_maps = []
    for c in range(8):
        b, r = c // 4, c % 4
        pos = segment_pos[b].astype(np.float32)
        seg = np.cumsum((segment_pos[b] == 0).astype(np.int64))

        qidx = np.concatenate(
            [np.arange(128) + 128 * _chunk_of_slot(r, p) for p in range(NSLOT)])
        kidx = np.arange(KVB) + KVB * r

        xqc = np.ascontiguousarray(x[b][qidx].T).astype(BF16).reshape(NW, 128, QBLK)
        xkvc = np.ascontiguousarray(x[b][kidx].T).astype(BF16).reshape(NW, 128, KVB)

        csq_, snq_ = cossin(pos[qidx])
        csk_, snk_ = cossin(pos[kidx])

        # packed masks: for key tile t, needed(t) slot blocks of [128k,128q]
        mask = np.zeros((128, MCOLS), dtype=BF16)
        for t in range(NTT):
            tk = np.arange(128) + 128 * t
            for p in range(_needed(t)):
                tq = qidx[p * 128:(p + 1) * 128]
                allow = (
                    (tk[:, None] <= tq[None, :])
                    & (tq[None, :] <= tk[:, None] + WINDOW)
                    & (seg[tk][:, None] == seg[tq][None, :])
                )
                mask[:, MOFF[t] + 128 * p: MOFF[t] + 128 * (p + 1)] = allow
        in_maps.append(dict(
            shared,
            xq=xqc,
            xkv=xkvc,
            csk=csk_,
            snk=snk_,
            csq=csq_,
            snq=snq_,
            msk=mask,
        ))
    return in_maps


def _assemble(results):
    out = np.empty((B, T, WIDTH), dtype=np.float32)
    for c, res in enumerate(results):
        b, r = c // 4, c % 4
        o = np.asarray(res["out"], dtype=np.float32)  # [NW, 128, QBLK]
        for p in range(NSLOT):
            ch = _chunk_of_slot(r, p)
            out[b, 128 * ch:128 * (ch + 1), :] = (
                o[:, :, 128 * p:128 * (p + 1)].transpose(2, 0, 1).reshape(128, WIDTH)
            )
    return out


def kernel(x, segment_pos, wq, wk, wv, w_final, b_final):
    from concourse.bass_utils import run_bass_kernel_spmd

    nc = get_nc()
    in_maps = _host_prepare(x, segment_pos, wq, wk, wv, w_final, b_final)
    res = run_bass_kernel_spmd(nc, in_maps, list(range(8)))
    return _assemble(res.results)


# revision 10
# speedup vs baseline: 1.4490x; 1.0322x over previous
"""Trainium2 Bass kernel for a local-attention block (MQA, RoPE, causal mask).

Reference computation (B=2, T=2048, WIDTH=2560, 10 q-heads, 1 kv-head,
head_dim=256, window=2048 => mask reduces to causal & same-segment):

    q = x @ wq.T ; k = x @ wk.T ; v = x @ wv.T
    q, k = rope(q), rope(k)
    probs = softmax(q k^T / 16 + mask)
    out = (probs @ v) @ w_final.T + b_final

Sharding: 8 cores = 2 batches x 4 ranks. Load-balanced causal split: rank r
owns the four 128-token query chunks {r, r+4, r+8, r+12}, placed in "slots"
ordered by decreasing causal coverage (16, 12, 8, 4 key tiles of 128). For
key tile t only the first needed(t) = 4 - t//4 slots are processed, so the
S/O matmuls use contiguous column prefixes of the slot-ordered Q buffer and
every core does identical work (SPMD) with no fully-masked tiles computed.

K/V projection is sharded: each core projects+ropes K/V for its own 512
tokens, then an AllGather over the 4 cores of a batch reconstructs the full
K^T / V in every core. The gather runs on the collective cores concurrently
with the Q projection.

Device layouts are "feature on partitions, tokens on free dim" so every
matmul contraction maps to the 128-partition axis with no on-device
transposes (except 128x128 PE transposes of the attention output).
"""

import sys

import numpy as np

for _p in ("/opt/trn_rl_repo", "/root/.axon_site/_ro/trn_rl_repo"):
    if _p not in sys.path:
        sys.path.insert(0, _p)

import ml_dtypes

BF16 = ml_dtypes.bfloat16

B, T, WIDTH = 2, 2048, 2560
NUM_HEADS, HEAD_DIM = 10, 256
WINDOW = 2048
MAX_WAVELENGTH = 10000.0
QBLK = 512              # query tokens per core (4 chunks of 128, slot order)
KVB = 512               # kv tokens projected per core
NW = WIDTH // 128       # 20 width stripes
NTT = T // 128          # 16 key token tiles
VROW = HEAD_DIM + 1     # v columns + ones column (denominator trick)
NSLOT = 4
NCOV = [16, 12, 8, 4]   # key-tile coverage per slot
# key tiles grouped into shared PSUM banks (equal needed() within a group)
TGROUPS = [[0], [1], [2], [3], [4], [5], [6], [7], [8, 9], [10, 11],
           [12, 13, 14, 15]]


def _needed(t):
    return 4 - t // 4


# mask/pt column offset of key tile t
MOFF = [0] * NTT
for _t in range(1, NTT):
    MOFF[_t] = MOFF[_t - 1] + 128 * _needed(_t - 1)
MCOLS = MOFF[-1] + 128 * _needed(NTT - 1)  # 5120

_NC_CACHE = {}


def _build_nc():
    """Build the (single, SPMD-uniform) Bass/Tile program."""
    import concourse.bass as bass  # noqa: F401
    import concourse.mybir as mybir
    import concourse.tile as tile
    from concourse import bacc
    from concourse.masks import make_identity

    fp32 = mybir.dt.float32
    bf16 = mybir.dt.bfloat16
    Exp = mybir.ActivationFunctionType.Exp
    Copy = mybir.ActivationFunctionType.Copy

    nc = bacc.Bacc("TRN2", target_bir_lowering=False, debug=False, num_devices=8)

    # ---- DRAM I/O ----
    xq = nc.dram_tensor("xq", [NW, 128, QBLK], bf16, kind="ExternalInput")
    xkv = nc.dram_tensor("xkv", [NW, 128, KVB], bf16, kind="ExternalInput")
    wq = nc.dram_tensor("wq", [NW, 128, WIDTH], bf16, kind="ExternalInput")
    wkr_d = nc.dram_tensor("wkr", [128, NW * HEAD_DIM], bf16, kind="ExternalInput")
    wvr_d = nc.dram_tensor("wvr", [128, NW * HEAD_DIM], bf16, kind="ExternalInput")
    wf = nc.dram_tensor("wf", [NW, 128, WIDTH], bf16, kind="ExternalInput")
    csk = nc.dram_tensor("csk", [64, KVB], fp32, kind="ExternalInput")
    snk = nc.dram_tensor("snk", [64, KVB], fp32, kind="ExternalInput")
    csq = nc.dram_tensor("csq", [64, QBLK], fp32, kind="ExternalInput")
    snq = nc.dram_tensor("snq", [64, QBLK], fp32, kind="ExternalInput")
    msk = nc.dram_tensor("msk", [128, MCOLS], bf16, kind="ExternalInput")
    bia = nc.dram_tensor("bia", [128, NW], fp32, kind="ExternalInput")
    out = nc.dram_tensor("out", [NW, 128, QBLK], fp32, kind="ExternalOutput")

    with tile.TileContext(nc) as tc:
        with (
            tc.tile_pool(name="res", bufs=1) as res,
            tc.tile_pool(name="wstr", bufs=2) as wstr,
            tc.tile_pool(name="ptp", bufs=4) as ptp,
            tc.tile_pool(name="enp", bufs=4) as enp,
            tc.tile_pool(name="tmp", bufs=4) as tmpp,
            tc.tile_pool(name="rcp", bufs=4) as rcpp,
            tc.tile_pool(name="outp", bufs=3) as outp,
            tc.tile_pool(name="dram", bufs=1, space="DRAM") as dram,
            tc.tile_pool(name="pp", bufs=2, space="PSUM") as pp,
            tc.tile_pool(name="stp", bufs=2, space="PSUM") as stp,
            tc.tile_pool(name="op", bufs=4, space="PSUM") as op,
        ):
            # ---- resident SBUF tiles ----
            xkvs = res.tile([128, NW * KVB], bf16, tag="xkvs")
            xqs = res.tile([128, NW * QBLK], bf16, tag="xqs")
            qtr = res.tile([128, NW * QBLK], bf16, tag="qtr")   # rope'd Q^T
            ktr0 = res.tile([128, T], bf16, tag="ktr0")         # rope'd K^T hd 0:128
            ktr1 = res.tile([128, T], bf16, tag="ktr1")         # K^T hd 128:256
            vsb = res.tile([128, NTT * VROW], bf16, tag="vsb")  # V tiles + ones col
            kvsh = res.tile([128, 2048], bf16, tag="kvsh")      # own K/V shard
            wkr = res.tile([128, NW * HEAD_DIM], bf16, tag="wkr")
            wvr = res.tile([128, NW * HEAD_DIM], bf16, tag="wvr")
            enct = res.tile([128, NW * QBLK], bf16, tag="enct")
            csk_s = res.tile([64, KVB], fp32, tag="csk")
            snk_s = res.tile([64, KVB], fp32, tag="snk")
            csq_s = res.tile([64, QBLK], fp32, tag="csq")
            snq_s = res.tile([64, QBLK], fp32, tag="snq")
            msk_s = res.tile([128, MCOLS], bf16, tag="msk")
            bia_s = res.tile([128, NW], fp32, tag="bia")
            ident = res.tile([128, 128], bf16, tag="ident")

            make_identity(nc, ident[:])

            kv_in = dram.tile([128, 2048], bf16, tag="kvi")
            kv_out = dram.tile([4, 128, 2048], bf16, tag="kvo")

            # ---- input DMAs (SP queue, in consumption order) ----
            nc.sync.dma_start(out=wkr[:], in_=wkr_d[:])
            nc.sync.dma_start(out=wvr[:], in_=wvr_d[:])
            for kc in range(4):  # split so the K/V k-loop starts early
                ks = slice(5 * kc, 5 * (kc + 1))
                nc.sync.dma_start(
                    out=xkvs[:].rearrange("p (k c) -> p k c", k=NW)[:, ks],
                    in_=xkv[:].rearrange("k p c -> p k c")[:, ks])
            nc.sync.dma_start(out=csk_s[:], in_=csk[:])
            nc.sync.dma_start(out=snk_s[:], in_=snk[:])
            nc.sync.dma_start(
                out=xqs[:].rearrange("p (k c) -> p k c", k=NW),
                in_=xq[:].rearrange("k p c -> p k c"))
            nc.sync.dma_start(out=csq_s[:], in_=csq[:])
            nc.sync.dma_start(out=snq_s[:], in_=snq[:])

            # ones columns of V (denominator of softmax via matmul)
            nc.gpsimd.memset(
                vsb[:].rearrange("p (n v) -> p n v", n=NTT)[:, :, HEAD_DIM:VROW],
                1.0)

            def rope_evict(ps, cs, sn, dst0, dst1):
                """dst0 = ps0*cos - ps1*sin ; dst1 = ps1*cos + ps0*sin.

                ps: [128, n] PSUM fp32; cs/sn: [64, n] SBUF fp32 tables;
                dst0/dst1: bf16 SBUF APs [64, n] (head-dim halves)."""
                n = cs.shape[-1]
                t0 = tmpp.tile([64, QBLK], fp32, tag="t0", name="t0")
                t1 = tmpp.tile([64, QBLK], fp32, tag="t1", name="t1")
                nc.vector.tensor_mul(t0[:, :n], ps[0:64, :], cs)
                nc.vector.tensor_mul(t1[:, :n], ps[64:128, :], sn)
                nc.vector.tensor_sub(dst0, t0[:, :n], t1[:, :n])
                t2 = tmpp.tile([64, QBLK], fp32, tag="t0", name="t2")
                t3 = tmpp.tile([64, QBLK], fp32, tag="t1", name="t3")
                nc.vector.tensor_mul(t2[:, :n], ps[64:128, :], cs)
                nc.vector.tensor_mul(t3[:, :n], ps[0:64, :], sn)
                nc.vector.tensor_add(dst1, t2[:, :n], t3[:, :n])

            # ---- K/V shard projection (own 512 tokens) ----
            # kvsh cols: [0:512] rope'd K^T hd0:128, [512:1024] K^T hd128:256,
            # [1024:2048] V tiles (4 x [128tok, 256hd])
            psk0 = pp.tile([128, KVB], fp32, tag="pp", name="psk0")
            psk1 = pp.tile([128, KVB], fp32, tag="pp", name="psk1")
            psv = [op.tile([128, HEAD_DIM], fp32, tag="o", name=f"psv{mt}")
                   for mt in range(4)]
            for k in range(NW):
                nc.tensor.matmul(
                    psk0[:], lhsT=wkr[:, k * HEAD_DIM: k * HEAD_DIM + 128],
                    rhs=xkvs[:, k * KVB:(k + 1) * KVB],
                    start=(k == 0), stop=(k == NW - 1))
                nc.tensor.matmul(
                    psk1[:], lhsT=wkr[:, k * HEAD_DIM + 128:(k + 1) * HEAD_DIM],
                    rhs=xkvs[:, k * KVB:(k + 1) * KVB],
                    start=(k == 0), stop=(k == NW - 1))
                for mt in range(4):
                    nc.tensor.matmul(
                        psv[mt][:],
                        lhsT=xkvs[:, k * KVB + mt * 128: k * KVB + (mt + 1) * 128],
                        rhs=wvr[:, k * HEAD_DIM:(k + 1) * HEAD_DIM],
                        start=(k == 0), stop=(k == NW - 1))
            rope_evict(psk0, csk_s[:], snk_s[:],
                       kvsh[0:64, 0:KVB], kvsh[64:128, 0:KVB])
            nc.scalar.copy(out=kvsh[:, KVB:2 * KVB], in_=psk1[:])
            for mt in range(4):
                nc.scalar.copy(
                    out=kvsh[:, 1024 + mt * HEAD_DIM: 1024 + (mt + 1) * HEAD_DIM],
                    in_=psv[mt][:])

            # ---- K/V all-gather across the 4 cores of this batch ----
            nc.gpsimd.dma_start(out=kv_in[:], in_=kvsh[:])
            nc.gpsimd.collective_compute(
                "AllGather",
                mybir.AluOpType.bypass,
                replica_groups=[[0, 1, 2, 3], [4, 5, 6, 7]],
                ins=[kv_in.opt()],
                outs=[kv_out.opt()],
            )
            nc.gpsimd.dma_start(
                out=ktr0[:].rearrange("p (r c) -> p r c", r=4),
                in_=kv_out[:, :, 0:KVB].rearrange("r p c -> p r c"))
            nc.gpsimd.dma_start(
                out=ktr1[:].rearrange("p (r c) -> p r c", r=4),
                in_=kv_out[:, :, KVB:2 * KVB].rearrange("r p c -> p r c"))
            for r4 in range(4):
                nc.gpsimd.dma_start(
                    out=vsb[:].rearrange("p (n v) -> p n v", n=NTT)[
                        :, 4 * r4:4 * (r4 + 1), 0:HEAD_DIM],
                    in_=kv_out[r4, :, 1024:2048].rearrange(
                        "p (t v) -> p t v", t=4))

            # ---- Q projection -> rope'd Q^T stripes [qdim, QBLK] ----
            # stripe m: qdim rows [128m, 128m+128) = head m//2, half m%2
            for m in range(NW):
                wq_m = wstr.tile([128, WIDTH], bf16, tag="w")
                nc.sync.dma_start(out=wq_m[:], in_=wq[m])
                ps = pp.tile([128, QBLK], fp32, tag="pp")
                for k in range(NW):
                    nc.tensor.matmul(
                        ps[:],
                        lhsT=wq_m[:, k * 128:(k + 1) * 128],
                        rhs=xqs[:, k * QBLK:(k + 1) * QBLK],
                        start=(k == 0),
                        stop=(k == NW - 1),
                    )
                dst = qtr[:, m * QBLK:(m + 1) * QBLK]
                if m % 2 == 0:  # rope half of the head dims
                    rope_evict(ps, csq_s[:], snq_s[:],
                               qtr[0:64, m * QBLK:(m + 1) * QBLK],
                               qtr[64:128, m * QBLK:(m + 1) * QBLK])
                else:           # passthrough half
                    nc.scalar.copy(out=dst, in_=ps[:])

            # masks + bias arrive behind the wq stripes, before attention
            nc.sync.dma_start(out=msk_s[:], in_=msk[:])
            nc.sync.dma_start(out=bia_s[:], in_=bia[:])

            # ---- attention (S^T layout: k on partitions, q on free dim) ----
            def evict_slot(h, p, o_tile):
                r = rcpp.tile([128, 1], fp32, tag="r")
                nc.vector.reciprocal(r[:], o_tile[:, HEAD_DIM:VROW])
                en = enp.tile([128, HEAD_DIM], bf16, tag="en")
                nc.scalar.activation(en[:], o_tile[:, 0:HEAD_DIM], Copy,
                                     scale=r[:])
                for hh in range(2):
                    tp = pp.tile([128, 128], bf16, tag="pp")
                    nc.tensor.matmul(
                        tp[:], lhsT=en[:, hh * 128:(hh + 1) * 128],
                        rhs=ident[:], is_transpose=True)
                    nc.vector.tensor_copy(
                        enct[:, (2 * h + hh) * QBLK + p * 128:
                             (2 * h + hh) * QBLK + (p + 1) * 128],
                        tp[:])

            # Software-pipelined over a flat (head, group) stream: the O
            # matmuls lag one group behind S/exp/mask so the Act+DVE latency
            # between S and O is never exposed on the tensor engine; the
            # eviction transposes lag one more group.
            all_groups = [(h, grp) for h in range(NUM_HEADS) for grp in TGROUPS]
            o_by_head = {}
            pts = {}

            def emit_s(i):
                h, grp = all_groups[i]
                if grp is TGROUPS[0]:
                    o_by_head[h] = [
                        op.tile([128, VROW], fp32, tag="o", name=f"o{h}_{p}")
                        for p in range(NSLOT)]
                nd = _needed(grp[0])
                gw = 128 * nd * len(grp)   # group column width
                st = stp.tile([128, QBLK], fp32, tag="st")
                for j, t in enumerate(grp):
                    cols = slice(j * 128 * nd, (j + 1) * 128 * nd)
                    nc.tensor.matmul(
                        st[:, cols], lhsT=ktr0[:, t * 128:(t + 1) * 128],
                        rhs=qtr[:, (2 * h) * QBLK:(2 * h) * QBLK + 128 * nd],
                        start=True, stop=False)
                    nc.tensor.matmul(
                        st[:, cols], lhsT=ktr1[:, t * 128:(t + 1) * 128],
                        rhs=qtr[:, (2 * h + 1) * QBLK:
                                 (2 * h + 1) * QBLK + 128 * nd],
                        start=False, stop=True)
                pt = ptp.tile([128, QBLK], bf16, tag="pt")
                # p = exp(s / sqrt(head_dim)), masked entries -> 0
                nc.scalar.activation(pt[:, :gw], st[:, :gw], Exp, scale=0.0625)
                nc.vector.tensor_mul(
                    pt[:, :gw], pt[:, :gw],
                    msk_s[:, MOFF[grp[0]]:MOFF[grp[0]] + gw])
                pts[i] = pt

            def emit_o(i):
                h, grp = all_groups[i]
                nd = _needed(grp[0])
                pt = pts.pop(i)
                for j, t in enumerate(grp):
                    for p in reversed(range(nd)):
                        nc.tensor.matmul(
                            o_by_head[h][p][:],
                            lhsT=pt[:, j * 128 * nd + p * 128:
                                    j * 128 * nd + (p + 1) * 128],
                            rhs=vsb[:, t * VROW:(t + 1) * VROW],
                            start=(t == 0),
                            stop=(t == NCOV[p] - 1),
                        )

            evq = []   # (h, p, en) awaiting their PE transposes

            def emit_evict_scale(i):
                h, grp = all_groups[i]
                for p in range(NSLOT):
                    if NCOV[p] - 1 != grp[-1]:
                        continue
                    o_tile = o_by_head[h][p]
                    r = rcpp.tile([128, 1], fp32, tag="r")
                    nc.vector.reciprocal(r[:], o_tile[:, HEAD_DIM:VROW])
                    en = enp.tile([128, HEAD_DIM], bf16, tag="en")
                    nc.scalar.activation(en[:], o_tile[:, 0:HEAD_DIM], Copy,
                                         scale=r[:])
                    evq.append((h, p, en))

            def emit_transposes():
                while evq:
                    h, p, en = evq.pop(0)
                    for hh in range(2):
                        tp = pp.tile([128, 128], bf16, tag="pp")
                        nc.tensor.matmul(
                            tp[:], lhsT=en[:, hh * 128:(hh + 1) * 128],
                            rhs=ident[:], is_transpose=True)
                        nc.vector.tensor_copy(
                            enct[:, (2 * h + hh) * QBLK + p * 128:
                                 (2 * h + hh) * QBLK + (p + 1) * 128],
                            tp[:])

            NG = len(all_groups)
            for i in range(NG):
                emit_s(i)
                if i >= 1:
                    emit_o(i - 1)
                    emit_transposes()       # drain evictions from <= i-2
                    emit_evict_scale(i - 1)
            emit_o(NG - 1)
            emit_transposes()
            emit_evict_scale(NG - 1)
            emit_transposes()

            # ---- final projection: out^T = wf @ enc^T + bias ----
            for m in range(NW):
                wf_m = wstr.tile([128, WIDTH], bf16, tag="w")
                nc.sync.dma_start(out=wf_m[:], in_=wf[m])
                ps = pp.tile([128, QBLK], fp32, tag="pp")
                for k in range(NW):
                    nc.tensor.matmul(
                        ps[:],
                        lhsT=wf_m[:, k * 128:(k + 1) * 128],
                        rhs=enct[:, k * QBLK:(k + 1) * QBLK],
                        start=(k == 0),
                        stop=(k == NW - 1),
                    )
                osb = outp.tile([128, QBLK], fp32, tag="osb")
                nc.vector.tensor_scalar_add(osb[:], ps[:], bia_s[:, m:m + 1])
                nc.sync.dma_start(out=out[m], in_=osb[:])

    if not nc.is_finalized():
        nc.finalize()  # bacc register allocation — required before walrus compile
    return nc


def get_nc():
    if "nc" not in _NC_CACHE:
        _NC_CACHE["nc"] = _build_nc()
    return _NC_CACHE["nc"]


def _chunk_of_slot(r, p):
    """Physical 128-token query chunk held by rank r's slot p."""
    return r + 12 - 4 * p


def _host_prepare(x, segment_pos, wq, wk, wv, w_final, b_final):
    """Build shared + per-core device input arrays."""
    x = np.asarray(x, dtype=np.float32)
    segment_pos = np.asarray(segment_pos)
    wq = np.asarray(wq, dtype=np.float32)
    wk = np.asarray(wk, dtype=np.float32)
    wv = np.asarray(wv, dtype=np.float32)
    w_final = np.asarray(w_final, dtype=np.float32)
    b_final = np.asarray(b_final, dtype=np.float32)

    def stripes_sq(w):  # [WIDTH, WIDTH] torch-Linear weight -> [20,128,WIDTH] w^T stripes
        wt = np.ascontiguousarray(w.T)
        return np.ascontiguousarray(
            wt.reshape(NW, 128, NW, 128).transpose(2, 1, 0, 3).reshape(NW, 128, WIDTH)
        ).astype(BF16)

    def packed_kv(w):  # [HEAD_DIM, WIDTH] -> [128, NW*HEAD_DIM] w^T stripes
        return np.ascontiguousarray(
            w.T.reshape(NW, 128, HEAD_DIM).transpose(1, 0, 2).reshape(
                128, NW * HEAD_DIM)
        ).astype(BF16)

    shared = {
        "wq": stripes_sq(wq),
        "wf": stripes_sq(w_final),
        "wkr": packed_kv(wk),
        "wvr": packed_kv(wv),
        "bia": np.ascontiguousarray(b_final.reshape(NW, 128).T).astype(np.float32),
    }

    inv_freq = (
        1.0 / MAX_WAVELENGTH ** (2.0 * np.arange(HEAD_DIM // 4, dtype=np.float32)
                                 / (HEAD_DIM // 2))
    ).astype(np.float32)

    def cossin(pos):
        ang = inv_freq[:, None] * pos[None, :].astype(np.float32)
        return (np.cos(ang).astype(np.float32), np.sin(ang).astype(np.float32))

    in_maps = []
    for c in range(8):
        b, r = c // 4, c % 4
        pos = segment_pos[b].astype(np.float32)
        seg = np.cumsum((segment_pos[b] == 0).astype(np.int64))

        qidx = np.concatenate(
            [np.arange(128) + 128 * _chunk_of_slot(r, p) for p in range(NSLOT)])
        kidx = np.arange(KVB) + KVB * r

        xqc = np.ascontiguousarray(x[b][qidx].T).astype(BF16).reshape(NW, 128, QBLK)
        xkvc = np.ascontiguousarray(x[b][kidx].T).astype(BF16).reshape(NW, 128, KVB)

        csq_, snq_ = cossin(pos[qidx])
        csk_, snk_ = cossin(pos[kidx])

        # packed masks: for key tile t, needed(t) slot blocks of [128k,128q]
        mask = np.zeros((128, MCOLS), dtype=BF16)
        for t in range(NTT):
            tk = np.arange(128) + 128 * t
            for p in range(_needed(t)):
                tq = qidx[p * 128:(p + 1) * 128]
                allow = (
                    (tk[:, None] <= tq[None, :])
                    & (tq[None, :] <= tk[:, None] + WINDOW)
                    & (seg[tk][:, None] == seg[tq][None, :])
                )
                mask[:, MOFF[t] + 128 * p: MOFF[t] + 128 * (p + 1)] = allow
        in_maps.append(dict(
            shared,
            xq=xqc,
            xkv=xkvc,
            csk=csk_,
            snk=snk_,
            csq=csq_,
            snq=snq_,
            msk=mask,
        ))
    return in_maps


def _assemble(results):
    out = np.empty((B, T, WIDTH), dtype=np.float32)
    for c, res in enumerate(results):
        b, r = c // 4, c % 4
        o = np.asarray(res["out"], dtype=np.float32)  # [NW, 128, QBLK]
        for p in range(NSLOT):
            ch = _chunk_of_slot(r, p)
            out[b, 128 * ch:128 * (ch + 1), :] = (
                o[:, :, 128 * p:128 * (p + 1)].transpose(2, 0, 1).reshape(128, WIDTH)
            )
    return out


def kernel(x, segment_pos, wq, wk, wv, w_final, b_final):
    from concourse.bass_utils import run_bass_kernel_spmd

    nc = get_nc()
    in_maps = _host_prepare(x, segment_pos, wq, wk, wv, w_final, b_final)
    res = run_bass_kernel_spmd(nc, in_maps, list(range(8)))
    return _assemble(res.results)


# revision 14
# speedup vs baseline: 1.4974x; 1.0334x over previous
"""Trainium2 Bass kernel for a local-attention block (MQA, RoPE, causal mask).

Reference computation (B=2, T=2048, WIDTH=2560, 10 q-heads, 1 kv-head,
head_dim=256, window=2048 => mask reduces to causal & same-segment):

    q = x @ wq.T ; k = x @ wk.T ; v = x @ wv.T
    q, k = rope(q), rope(k)
    probs = softmax(q k^T / 16 + mask)
    out = (probs @ v) @ w_final.T + b_final

Sharding: 8 cores = 2 batches x 4 ranks. Load-balanced causal split: rank r
owns the four 128-token query chunks {r, r+4, r+8, r+12}, placed in "slots"
ordered by decreasing causal coverage (16, 12, 8, 4 key tiles of 128). For
key tile t only the first needed(t) = 4 - t//4 slots are processed, so the
S/O matmuls use contiguous column prefixes of the slot-ordered Q buffer and
every core does identical work (SPMD) with no fully-masked tiles computed.

K/V projection is sharded: each core projects+ropes K/V for its own 512
tokens, then an AllGather over the 4 cores of a batch reconstructs the full
K^T / V in every core. The gather runs on the collective cores concurrently
with the Q projection.

Device layouts are "feature on partitions, tokens on free dim" so every
matmul contraction maps to the 128-partition axis with no on-device
transposes (except 128x128 PE transposes of the attention output).
"""

import sys

import numpy as np

for _p in ("/opt/trn_rl_repo", "/root/.axon_site/_ro/trn_rl_repo"):
    if _p not in sys.path:
        sys.path.insert(0, _p)

import ml_dtypes

BF16 = ml_dtypes.bfloat16

B, T, WIDTH = 2, 2048, 2560
NUM_HEADS, HEAD_DIM = 10, 256
WINDOW = 2048
MAX_WAVELENGTH = 10000.0
QBLK = 512              # query tokens per core (4 chunks of 128, slot order)
KVB = 512               # kv tokens projected per core
NW = WIDTH // 128       # 20 width stripes
NTT = T // 128          # 16 key token tiles
VROW = HEAD_DIM + 1     # v columns + ones column (denominator trick)
NSLOT = 4
NCOV = [16, 12, 8, 4]   # key-tile coverage per slot
# key tiles grouped into shared PSUM banks (equal needed() within a group)
TGROUPS = [[0], [1], [2], [3], [4], [5], [6], [7], [8, 9], [10, 11],
           [12, 13, 14, 15]]


def _needed(t):
    return 4 - t // 4


# mask/pt column offset of key tile t
MOFF = [0] * NTT
for _t in range(1, NTT):
    MOFF[_t] = MOFF[_t - 1] + 128 * _needed(_t - 1)
MCOLS = MOFF[-1] + 128 * _needed(NTT - 1)  # 5120

_NC_CACHE = {}


def _build_nc():
    """Build the (single, SPMD-uniform) Bass/Tile program."""
    import concourse.bass as bass  # noqa: F401
    import concourse.mybir as mybir
    import concourse.tile as tile
    from concourse import bacc
    from concourse.masks import make_identity

    fp32 = mybir.dt.float32
    bf16 = mybir.dt.bfloat16
    Exp = mybir.ActivationFunctionType.Exp
    Copy = mybir.ActivationFunctionType.Copy

    nc = bacc.Bacc("TRN2", target_bir_lowering=False, debug=False, num_devices=8)

    # ---- DRAM I/O ----
    xq = nc.dram_tensor("xq", [NW, 128, QBLK], bf16, kind="ExternalInput")
    xkv = nc.dram_tensor("xkv", [NW, 128, KVB], bf16, kind="ExternalInput")
    wq = nc.dram_tensor("wq", [NW, 128, WIDTH], bf16, kind="ExternalInput")
    wkr_d = nc.dram_tensor("wkr", [128, NW * HEAD_DIM], bf16, kind="ExternalInput")
    wvr_d = nc.dram_tensor("wvr", [128, NW * HEAD_DIM], bf16, kind="ExternalInput")
    wf = nc.dram_tensor("wf", [NW, 128, WIDTH], bf16, kind="ExternalInput")
    csk = nc.dram_tensor("csk", [64, KVB], fp32, kind="ExternalInput")
    snk = nc.dram_tensor("snk", [64, KVB], fp32, kind="ExternalInput")
    csq = nc.dram_tensor("csq", [64, QBLK], fp32, kind="ExternalInput")
    snq = nc.dram_tensor("snq", [64, QBLK], fp32, kind="ExternalInput")
    msk = nc.dram_tensor("msk", [128, MCOLS], bf16, kind="ExternalInput")
    bia = nc.dram_tensor("bia", [128, NW], fp32, kind="ExternalInput")
    out = nc.dram_tensor("out", [NW, 128, QBLK], fp32, kind="ExternalOutput")

    with tile.TileContext(nc) as tc:
        with (
            tc.tile_pool(name="res", bufs=1) as res,
            tc.tile_pool(name="wstr", bufs=3) as wstr,
            tc.tile_pool(name="ptp", bufs=4) as ptp,
            tc.tile_pool(name="enp", bufs=4) as enp,
            tc.tile_pool(name="tmp", bufs=4) as tmpp,
            tc.tile_pool(name="rcp", bufs=4) as rcpp,
            tc.tile_pool(name="outp", bufs=3) as outp,
            tc.tile_pool(name="dram", bufs=1, space="DRAM") as dram,
            tc.tile_pool(name="pp", bufs=2, space="PSUM") as pp,
            tc.tile_pool(name="stp", bufs=2, space="PSUM") as stp,
            tc.tile_pool(name="op", bufs=4, space="PSUM") as op,
        ):
            # ---- resident SBUF tiles ----
            xkvs = res.tile([128, NW * KVB], bf16, tag="xkvs")
            xqs = res.tile([128, NW * QBLK], bf16, tag="xqs")
            qtr = res.tile([128, NW * QBLK], bf16, tag="qtr")   # rope'd Q^T
            ktr0 = res.tile([128, T], bf16, tag="ktr0")         # rope'd K^T hd 0:128
            ktr1 = res.tile([128, T], bf16, tag="ktr1")         # K^T hd 128:256
            vsb = res.tile([128, NTT * VROW], bf16, tag="vsb")  # V tiles + ones col
            kvsh = res.tile([128, 2048], bf16, tag="kvsh")      # own K/V shard
            wkr = res.tile([128, NW * HEAD_DIM], bf16, tag="wkr")
            wvr = res.tile([128, NW * HEAD_DIM], bf16, tag="wvr")
            enct = res.tile([128, NW * QBLK], bf16, tag="enct")
            csk_s = res.tile([64, KVB], fp32, tag="csk")
            snk_s = res.tile([64, KVB], fp32, tag="snk")
            csq_s = res.tile([64, QBLK], fp32, tag="csq")
            snq_s = res.tile([64, QBLK], fp32, tag="snq")
            msk_s = res.tile([128, MCOLS], bf16, tag="msk")
            bia_s = res.tile([128, NW], fp32, tag="bia")
            ident = res.tile([128, 128], bf16, tag="ident")

            make_identity(nc, ident[:])

            kv_in = dram.tile([128, 2048], bf16, tag="kvi")
            kv_out = dram.tile([4, 128, 2048], bf16, tag="kvo")

            # ---- input DMAs (SP queue, in consumption order) ----
            # wkr/wvr/xkv stream in 4 interleaved chunks of 5 width-stripes
            # so the K/V projection starts as soon as chunk 0 lands.
            for kc in range(4):
                ks = slice(5 * kc, 5 * (kc + 1))
                nc.sync.dma_start(
                    out=wkr[:, 5 * kc * HEAD_DIM:5 * (kc + 1) * HEAD_DIM],
                    in_=wkr_d[:, 5 * kc * HEAD_DIM:5 * (kc + 1) * HEAD_DIM])
                nc.sync.dma_start(
                    out=wvr[:, 5 * kc * HEAD_DIM:5 * (kc + 1) * HEAD_DIM],
                    in_=wvr_d[:, 5 * kc * HEAD_DIM:5 * (kc + 1) * HEAD_DIM])
                nc.sync.dma_start(
                    out=xkvs[:].rearrange("p (k c) -> p k c", k=NW)[:, ks],
                    in_=xkv[:].rearrange("k p c -> p k c")[:, ks])
            nc.sync.dma_start(out=csk_s[:], in_=csk[:])
            nc.sync.dma_start(out=snk_s[:], in_=snk[:])

            wq_tiles = {}

            def load_w(src, m):
                t = wstr.tile([128, WIDTH], bf16, tag="w")
                nc.sync.dma_start(out=t[:], in_=src[m])
                wq_tiles[m] = t

            load_w(wq, 0)
            for kc in range(4):
                ks = slice(5 * kc, 5 * (kc + 1))
                nc.sync.dma_start(
                    out=xqs[:].rearrange("p (k c) -> p k c", k=NW)[:, ks],
                    in_=xq[:].rearrange("k p c -> p k c")[:, ks])
            nc.sync.dma_start(out=csq_s[:], in_=csq[:])
            nc.sync.dma_start(out=snq_s[:], in_=snq[:])
            load_w(wq, 1)

            # ones columns of V (denominator of softmax via matmul)
            nc.gpsimd.memset(
                vsb[:].rearrange("p (n v) -> p n v", n=NTT)[:, :, HEAD_DIM:VROW],
                1.0)

            def rope_evict(ps, cs, sn, dst0, dst1):
                """dst0 = ps0*cos - ps1*sin ; dst1 = ps1*cos + ps0*sin.

                ps: [128, n] PSUM fp32; cs/sn: [64, n] SBUF fp32 tables;
                dst0/dst1: bf16 SBUF APs [64, n] (head-dim halves)."""
                n = cs.shape[-1]
                t0 = tmpp.tile([64, QBLK], fp32, tag="t0", name="t0")
                t1 = tmpp.tile([64, QBLK], fp32, tag="t1", name="t1")
                nc.vector.tensor_mul(t0[:, :n], ps[0:64, :], cs)
                nc.vector.tensor_mul(t1[:, :n], ps[64:128, :], sn)
                nc.vector.tensor_sub(dst0, t0[:, :n], t1[:, :n])
                t2 = tmpp.tile([64, QBLK], fp32, tag="t0", name="t2")
                t3 = tmpp.tile([64, QBLK], fp32, tag="t1", name="t3")
                nc.vector.tensor_mul(t2[:, :n], ps[64:128, :], cs)
                nc.vector.tensor_mul(t3[:, :n], ps[0:64, :], sn)
                nc.vector.tensor_add(dst1, t2[:, :n], t3[:, :n])

            # ---- K/V shard projection (own 512 tokens) ----
            # kvsh cols: [0:512] rope'd K^T hd0:128, [512:1024] K^T hd128:256,
            # [1024:2048] V tiles (4 x [128tok, 256hd])
            psk0 = pp.tile([128, KVB], fp32, tag="pp", name="psk0")
            psk1 = pp.tile([128, KVB], fp32, tag="pp", name="psk1")
            psv = [op.tile([128, HEAD_DIM], fp32, tag="o", name=f"psv{mt}")
                   for mt in range(4)]
            for k in range(NW):
                nc.tensor.matmul(
                    psk0[:], lhsT=wkr[:, k * HEAD_DIM: k * HEAD_DIM + 128],
                    rhs=xkvs[:, k * KVB:(k + 1) * KVB],
                    start=(k == 0), stop=(k == NW - 1))
                nc.tensor.matmul(
                    psk1[:], lhsT=wkr[:, k * HEAD_DIM + 128:(k + 1) * HEAD_DIM],
                    rhs=xkvs[:, k * KVB:(k + 1) * KVB],
                    start=(k == 0), stop=(k == NW - 1))
                for mt in range(4):
                    nc.tensor.matmul(
                        psv[mt][:],
                        lhsT=xkvs[:, k * KVB + mt * 128: k * KVB + (mt + 1) * 128],
                        rhs=wvr[:, k * HEAD_DIM:(k + 1) * HEAD_DIM],
                        start=(k == 0), stop=(k == NW - 1))
            rope_evict(psk0, csk_s[:], snk_s[:],
                       kvsh[0:64, 0:KVB], kvsh[64:128, 0:KVB])
            nc.scalar.copy(out=kvsh[:, KVB:2 * KVB], in_=psk1[:])
            for mt in range(4):
                nc.scalar.copy(
                    out=kvsh[:, 1024 + mt * HEAD_DIM: 1024 + (mt + 1) * HEAD_DIM],
                    in_=psv[mt][:])

            # ---- K/V all-gather across the 4 cores of this batch ----
            nc.gpsimd.dma_start(out=kv_in[:], in_=kvsh[:])
            nc.gpsimd.collective_compute(
                "AllGather",
                mybir.AluOpType.bypass,
                replica_groups=[[0, 1, 2, 3], [4, 5, 6, 7]],
                ins=[kv_in.opt()],
                outs=[kv_out.opt()],
            )
            nc.gpsimd.dma_start(
                out=ktr0[:].rearrange("p (r c) -> p r c", r=4),
                in_=kv_out[:, :, 0:KVB].rearrange("r p c -> p r c"))
            nc.gpsimd.dma_start(
                out=ktr1[:].rearrange("p (r c) -> p r c", r=4),
                in_=kv_out[:, :, KVB:2 * KVB].rearrange("r p c -> p r c"))
            for r4 in range(4):
                nc.gpsimd.dma_start(
                    out=vsb[:].rearrange("p (n v) -> p n v", n=NTT)[
                        :, 4 * r4:4 * (r4 + 1), 0:HEAD_DIM],
                    in_=kv_out[r4, :, 1024:2048].rearrange(
                        "p (t v) -> p t v", t=4))

            # ---- Q projection -> rope'd Q^T stripes [qdim, QBLK] ----
            # stripe m: qdim rows [128m, 128m+128) = head m//2, half m%2
            for m in range(NW):
                if m + 2 < NW:
                    load_w(wq, m + 2)
                wq_m = wq_tiles.pop(m)
                ps = pp.tile([128, QBLK], fp32, tag="pp")
                for k in range(NW):
                    nc.tensor.matmul(
                        ps[:],
                        lhsT=wq_m[:, k * 128:(k + 1) * 128],
                        rhs=xqs[:, k * QBLK:(k + 1) * QBLK],
                        start=(k == 0),
                        stop=(k == NW - 1),
                    )
                dst = qtr[:, m * QBLK:(m + 1) * QBLK]
                if m % 2 == 0:  # rope half of the head dims
                    rope_evict(ps, csq_s[:], snq_s[:],
                               qtr[0:64, m * QBLK:(m + 1) * QBLK],
                               qtr[64:128, m * QBLK:(m + 1) * QBLK])
                else:           # passthrough half
                    nc.scalar.copy(out=dst, in_=ps[:])

            # masks + bias arrive behind the wq stripes, before attention
            nc.sync.dma_start(out=msk_s[:], in_=msk[:])
            nc.sync.dma_start(out=bia_s[:], in_=bia[:])

            # ---- attention (S^T layout: k on partitions, q on free dim) ----
            def evict_slot(h, p, o_tile):
                r = rcpp.tile([128, 1], fp32, tag="r")
                nc.vector.reciprocal(r[:], o_tile[:, HEAD_DIM:VROW])
                en = enp.tile([128, HEAD_DIM], bf16, tag="en")
                nc.scalar.activation(en[:], o_tile[:, 0:HEAD_DIM], Copy,
                                     scale=r[:])
                for hh in range(2):
                    tp = pp.tile([128, 128], bf16, tag="pp")
                    nc.tensor.matmul(
                        tp[:], lhsT=en[:, hh * 128:(hh + 1) * 128],
                        rhs=ident[:], is_transpose=True)
                    nc.vector.tensor_copy(
                        enct[:, (2 * h + hh) * QBLK + p * 128:
                             (2 * h + hh) * QBLK + (p + 1) * 128],
                        tp[:])

            # Software-pipelined over a flat (head, group) stream: the O
            # matmuls lag one group behind S/exp/mask so the Act+DVE latency
            # between S and O is never exposed on the tensor engine; the
            # eviction transposes lag one more group.
            all_groups = [(h, grp) for h in range(NUM_HEADS) for grp in TGROUPS]
            o_by_head = {}
            pts = {}

            def emit_s(i):
                h, grp = all_groups[i]
                if grp is TGROUPS[0]:
                    o_by_head[h] = [
                        op.tile([128, VROW], fp32, tag="o", name=f"o{h}_{p}")
                        for p in range(NSLOT)]
                nd = _needed(grp[0])
                gw = 128 * nd * len(grp)   # group column width
                st = stp.tile([128, QBLK], fp32, tag="st")
                for j, t in enumerate(grp):
                    cols = slice(j * 128 * nd, (j + 1) * 128 * nd)
                    nc.tensor.matmul(
                        st[:, cols], lhsT=ktr0[:, t * 128:(t + 1) * 128],
                        rhs=qtr[:, (2 * h) * QBLK:(2 * h) * QBLK + 128 * nd],
                        start=True, stop=False)
                    nc.tensor.matmul(
                        st[:, cols], lhsT=ktr1[:, t * 128:(t + 1) * 128],
                        rhs=qtr[:, (2 * h + 1) * QBLK:
                                 (2 * h + 1) * QBLK + 128 * nd],
                        start=False, stop=True)
                pt = ptp.tile([128, QBLK], bf16, tag="pt")
                # p = exp(s / sqrt(head_dim)), masked entries -> 0
                nc.scalar.activation(pt[:, :gw], st[:, :gw], Exp, scale=0.0625)
                nc.vector.tensor_mul(
                    pt[:, :gw], pt[:, :gw],
                    msk_s[:, MOFF[grp[0]]:MOFF[grp[0]] + gw])
                pts[i] = pt

            def emit_o(i):
                h, grp = all_groups[i]
                nd = _needed(grp[0])
                pt = pts.pop(i)
                for j, t in enumerate(grp):
                    for p in reversed(range(nd)):
                        nc.tensor.matmul(
                            o_by_head[h][p][:],
                            lhsT=pt[:, j * 128 * nd + p * 128:
                                    j * 128 * nd + (p + 1) * 128],
                            rhs=vsb[:, t * VROW:(t + 1) * VROW],
                            start=(t == 0),
                            stop=(t == NCOV[p] - 1),
                        )

            evq = []   # (h, p, en) awaiting their PE transposes

            def emit_evict_scale(i):
                h, grp = all_groups[i]
                for p in range(NSLOT):
                    if NCOV[p] - 1 != grp[-1]:
                        continue
                    o_tile = o_by_head[h][p]
                    r = rcpp.tile([128, 1], fp32, tag="r")
                    nc.vector.reciprocal(r[:], o_tile[:, HEAD_DIM:VROW])
                    en = enp.tile([128, HEAD_DIM], bf16, tag="en")
                    nc.scalar.activation(en[:], o_tile[:, 0:HEAD_DIM], Copy,
                                         scale=r[:])
                    evq.append((h, p, en))

            def emit_transposes():
                while evq:
                    h, p, en = evq.pop(0)
                    for hh in range(2):
                        tp = pp.tile([128, 128], bf16, tag="pp")
                        nc.tensor.matmul(
                            tp[:], lhsT=en[:, hh * 128:(hh + 1) * 128],
                            rhs=ident[:], is_transpose=True)
                        nc.vector.tensor_copy(
                            enct[:, (2 * h + hh) * QBLK + p * 128:
                                 (2 * h + hh) * QBLK + (p + 1) * 128],
                            tp[:])

            NG = len(all_groups)
            for i in range(NG):
                emit_s(i)
                if i >= 1:
                    emit_o(i - 1)
                    emit_transposes()       # drain evictions from <= i-2
                    emit_evict_scale(i - 1)
            emit_o(NG - 1)
            emit_transposes()
            emit_evict_scale(NG - 1)
            emit_transposes()

            # ---- final projection: out^T = wf @ enc^T + bias ----
            for m in range(NW):
                wf_m = wstr.tile([128, WIDTH], bf16, tag="w")
                nc.sync.dma_start(out=wf_m[:], in_=wf[m])
                ps = pp.tile([128, QBLK], fp32, tag="pp")
                for k in range(NW):
                    nc.tensor.matmul(
                        ps[:],
                        lhsT=wf_m[:, k * 128:(k + 1) * 128],
                        rhs=enct[:, k * QBLK:(k + 1) * QBLK],
                        start=(k == 0),
                        stop=(k == NW - 1),
                    )
                osb = outp.tile([128, QBLK], fp32, tag="osb")
                if m < NW - 1:
                    nc.vector.tensor_scalar_add(osb[:], ps[:], bia_s[:, m:m + 1])
                    nc.sync.dma_start(out=out[m], in_=osb[:])
                else:
                    # split the last stripe so the tail drain is short
                    for j4 in range(4):
                        cj = slice(128 * j4, 128 * (j4 + 1))
                        nc.vector.tensor_scalar_add(
                            osb[:, cj], ps[:, cj], bia_s[:, m:m + 1])
                        nc.sync.dma_start(out=out[m][:, cj], in_=osb[:, cj])

    if not nc.is_finalized():
        nc.finalize()  # bacc register allocation — required before walrus compile
    return nc


def get_nc():
    if "nc" not in _NC_CACHE:
        _NC_CACHE["nc"] = _build_nc()
    return _NC_CACHE["nc"]


def _chunk_of_slot(r, p):
    """Physical 128-token query chunk held by rank r's slot p."""
    return r + 12 - 4 * p


def _host_prepare(x, segment_pos, wq, wk, wv, w_final, b_final):
    """Build shared + per-core device input arrays."""
    x = np.asarray(x, dtype=np.float32)
    segment_pos = np.asarray(segment_pos)
    wq = np.asarray(wq, dtype=np.float32)
    wk = np.asarray(wk, dtype=np.float32)
    wv = np.asarray(wv, dtype=np.float32)
    w_final = np.asarray(w_final, dtype=np.float32)
    b_final = np.asarray(b_final, dtype=np.float32)

    def stripes_sq(w):  # [WIDTH, WIDTH] torch-Linear weight -> [20,128,WIDTH] w^T stripes
        wt = np.ascontiguousarray(w.T)
        return np.ascontiguousarray(
            wt.reshape(NW, 128, NW, 128).transpose(2, 1, 0, 3).reshape(NW, 128, WIDTH)
        ).astype(BF16)

    def packed_kv(w):  # [HEAD_DIM, WIDTH] -> [128, NW*HEAD_DIM] w^T stripes
        return np.ascontiguousarray(
            w.T.reshape(NW, 128, HEAD_DIM).transpose(1, 0, 2).reshape(
                128, NW * HEAD_DIM)
        ).astype(BF16)

    shared = {
        "wq": stripes_sq(wq),
        "wf": stripes_sq(w_final),
        "wkr": packed_kv(wk),
        "wvr": packed_kv(wv),
        "bia": np.ascontiguousarray(b_final.reshape(NW, 128).T).astype(np.float32),
    }

    inv_freq = (
        1.0 / MAX_WAVELENGTH ** (2.0 * np.arange(HEAD_DIM // 4, dtype=np.float32)
                                 / (HEAD_DIM // 2))
    ).astype(np.float32)

    def cossin(pos):
        ang = inv_freq[:, None] * pos[None, :].astype(np.float32)
        return (np.cos(ang).astype(np.float32), np.sin(ang).astype(np.float32))

    in_maps = []
    for c in range(8):
        b, r = c // 4, c % 4
        pos = segment_pos[b].astype(np.float32)
        seg = np.cumsum((segment_pos[b] == 0).astype(np.int64))

        qidx = np.concatenate(
            [np.arange(128) + 128 * _chunk_of_slot(r, p) for p in range(NSLOT)])
        kidx = np.arange(KVB) + KVB * r

        xqc = np.ascontiguousarray(x[b][qidx].T).astype(BF16).reshape(NW, 128, QBLK)
        xkvc = np.ascontiguousarray(x[b][kidx].T).astype(BF16).reshape(NW, 128, KVB)

        csq_, snq_ = cossin(pos[qidx])
        csk_, snk_ = cossin(pos[kidx])

        # packed masks: for key tile t, needed(t) slot blocks of [128k,128q]
        mask = np.zeros((128, MCOLS), dtype=BF16)
        for t in range(NTT):
            tk = np.arange(128) + 128 * t
            for p in range(_needed(t)):
                tq = qidx[p * 128:(p + 1) * 128]
                allow = (
                    (tk[:, None] <= tq[None, :])
                    & (tq[None, :] <= tk[:, None] + WINDOW)
                    & (seg[tk][:, None] == seg[tq][None, :])
                )
                mask[:, MOFF[t] + 128 * p: MOFF[t] + 128 * (p + 1)] = allow
        in_maps.append(dict(
            shared,
            xq=xqc,
            xkv=xkvc,
            csk=csk_,
            snk=snk_,
            csq=csq_,
            snq=snq_,
            msk=mask,
        ))
    return in_maps


def _assemble(results):
    out = np.empty((B, T, WIDTH), dtype=np.float32)
    for c, res in enumerate(results):
        b, r = c // 4, c % 4
        o = np.asarray(res["out"], dtype=np.float32)  # [NW, 128, QBLK]
        for p in range(NSLOT):
            ch = _chunk_of_slot(r, p)
            out[b, 128 * ch:128 * (ch + 1), :] = (
                o[:, :, 128 * p:128 * (p + 1)].transpose(2, 0, 1).reshape(128, WIDTH)
            )
    return out


def kernel(x, segment_pos, wq, wk, wv, w_final, b_final):
    from concourse.bass_utils import run_bass_kernel_spmd

    nc = get_nc()
    in_maps = _host_prepare(x, segment_pos, wq, wk, wv, w_final, b_final)
    res = run_bass_kernel_spmd(nc, in_maps, list(range(8)))
    return _assemble(res.results)


# revision 17
# speedup vs baseline: 1.5541x; 1.0379x over previous
"""Trainium2 Bass kernel for a local-attention block (MQA, RoPE, causal mask).

Reference computation (B=2, T=2048, WIDTH=2560, 10 q-heads, 1 kv-head,
head_dim=256, window=2048 => mask reduces to causal & same-segment):

    q = x @ wq.T ; k = x @ wk.T ; v = x @ wv.T
    q, k = rope(q), rope(k)
    probs = softmax(q k^T / 16 + mask)
    out = (probs @ v) @ w_final.T + b_final

Sharding: 8 cores = 2 batches x 4 ranks. Load-balanced causal split: rank r
owns the four 128-token query chunks {r, r+4, r+8, r+12}, placed in "slots"
ordered by decreasing causal coverage (16, 12, 8, 4 key tiles of 128). For
key tile t only the first needed(t) = 4 - t//4 slots are processed, so the
S/O matmuls use contiguous column prefixes of the slot-ordered Q buffer and
every core does identical work (SPMD) with no fully-masked tiles computed.

K/V projection is sharded: each core projects+ropes K/V for its own 512
tokens, then an AllGather over the 4 cores of a batch reconstructs the full
K^T / V in every core. The gather runs on the collective cores concurrently
with the Q projection.

Device layouts are "feature on partitions, tokens on free dim" so every
matmul contraction maps to the 128-partition axis with no on-device
transposes (except 128x128 PE transposes of the attention output).
"""

import sys

import numpy as np

for _p in ("/opt/trn_rl_repo", "/root/.axon_site/_ro/trn_rl_repo"):
    if _p not in sys.path:
        sys.path.insert(0, _p)

import ml_dtypes

BF16 = ml_dtypes.bfloat16

B, T, WIDTH = 2, 2048, 2560
NUM_HEADS, HEAD_DIM = 10, 256
WINDOW = 2048
MAX_WAVELENGTH = 10000.0
QBLK = 512              # query tokens per core (4 chunks of 128, slot order)
KVB = 512               # kv tokens projected per core
NW = WIDTH // 128       # 20 width stripes
NTT = T // 128          # 16 key token tiles
VROW = HEAD_DIM + 1     # v columns + ones column (denominator trick)
NSLOT = 4
NCOV = [16, 12, 8, 4]   # key-tile coverage per slot
# key tiles grouped into shared PSUM banks (equal needed() within a group)
TGROUPS = [[0], [1], [2], [3], [4], [5], [6], [7], [8, 9], [10, 11],
           [12, 13, 14, 15]]


def _needed(t):
    return 4 - t // 4


# mask/pt column offset of key tile t
MOFF = [0] * NTT
for _t in range(1, NTT):
    MOFF[_t] = MOFF[_t - 1] + 128 * _needed(_t - 1)
MCOLS = MOFF[-1] + 128 * _needed(NTT - 1)  # 5120

_NC_CACHE = {}


def _build_nc():
    """Build the (single, SPMD-uniform) Bass/Tile program."""
    import concourse.bass as bass  # noqa: F401
    import concourse.mybir as mybir
    import concourse.tile as tile
    from concourse import bacc
    from concourse.masks import make_identity

    fp32 = mybir.dt.float32
    bf16 = mybir.dt.bfloat16
    Exp = mybir.ActivationFunctionType.Exp
    Copy = mybir.ActivationFunctionType.Copy

    nc = bacc.Bacc("TRN2", target_bir_lowering=False, debug=False, num_devices=8)

    # ---- DRAM I/O ----
    xq = nc.dram_tensor("xq", [NW, 128, QBLK], bf16, kind="ExternalInput")
    xkv = nc.dram_tensor("xkv", [NW, 128, KVB], bf16, kind="ExternalInput")
    wq = nc.dram_tensor("wq", [NW, 128, WIDTH], bf16, kind="ExternalInput")
    wkr_d = nc.dram_tensor("wkr", [128, NW * HEAD_DIM], bf16, kind="ExternalInput")
    wvr_d = nc.dram_tensor("wvr", [128, NW * HEAD_DIM], bf16, kind="ExternalInput")
    wf = nc.dram_tensor("wf", [NW, 128, WIDTH], bf16, kind="ExternalInput")
    csk = nc.dram_tensor("csk", [64, KVB], fp32, kind="ExternalInput")
    snk = nc.dram_tensor("snk", [64, KVB], fp32, kind="ExternalInput")
    csq = nc.dram_tensor("csq", [64, QBLK], fp32, kind="ExternalInput")
    snq = nc.dram_tensor("snq", [64, QBLK], fp32, kind="ExternalInput")
    msk = nc.dram_tensor("msk", [128, MCOLS], bf16, kind="ExternalInput")
    bia = nc.dram_tensor("bia", [128, NW], fp32, kind="ExternalInput")
    out = nc.dram_tensor("out", [NW, 128, QBLK], fp32, kind="ExternalOutput")

    with tile.TileContext(nc) as tc:
        with (
            tc.tile_pool(name="res", bufs=1) as res,
            tc.tile_pool(name="wstr", bufs=3) as wstr,
            tc.tile_pool(name="ptp", bufs=6) as ptp,
            tc.tile_pool(name="enp", bufs=4) as enp,
            tc.tile_pool(name="tmp", bufs=4) as tmpp,
            tc.tile_pool(name="rcp", bufs=4) as rcpp,
            tc.tile_pool(name="outp", bufs=3) as outp,
            tc.tile_pool(name="dram", bufs=1, space="DRAM") as dram,
            tc.tile_pool(name="pp", bufs=2, space="PSUM") as pp,
            tc.tile_pool(name="stp", bufs=2, space="PSUM") as stp,
            tc.tile_pool(name="op", bufs=4, space="PSUM") as op,
        ):
            # ---- resident SBUF tiles ----
            xkvs = res.tile([128, NW * KVB], bf16, tag="xkvs")
            xqs = res.tile([128, NW * QBLK], bf16, tag="xqs")
            qtr = res.tile([128, NW * QBLK], bf16, tag="qtr")   # rope'd Q^T
            ktr0 = res.tile([128, T], bf16, tag="ktr0")         # rope'd K^T hd 0:128
            ktr1 = res.tile([128, T], bf16, tag="ktr1")         # K^T hd 128:256
            vsb = res.tile([128, NTT * VROW], bf16, tag="vsb")  # V tiles + ones col
            kvsh = res.tile([128, 2048], bf16, tag="kvsh")      # own K/V shard
            wkr = res.tile([128, NW * HEAD_DIM], bf16, tag="wkr")
            wvr = res.tile([128, NW * HEAD_DIM], bf16, tag="wvr")
            enct = res.tile([128, NW * QBLK], bf16, tag="enct")
            csk_s = res.tile([64, KVB], fp32, tag="csk")
            snk_s = res.tile([64, KVB], fp32, tag="snk")
            csq_s = res.tile([64, QBLK], fp32, tag="csq")
            snq_s = res.tile([64, QBLK], fp32, tag="snq")
            msk_s = res.tile([128, MCOLS], bf16, tag="msk")
            bia_s = res.tile([128, NW], fp32, tag="bia")
            ident = res.tile([128, 128], bf16, tag="ident")

            make_identity(nc, ident[:])

            kv_in = dram.tile([128, 2048], bf16, tag="kvi")
            kv_out = dram.tile([4, 128, 2048], bf16, tag="kvo")

            # ---- input DMAs (SP queue, in consumption order) ----
            # wkr/wvr/xkv stream in 4 interleaved chunks of 5 width-stripes
            # so the K/V projection starts as soon as chunk 0 lands.
            for kc in range(4):
                ks = slice(5 * kc, 5 * (kc + 1))
                nc.sync.dma_start(
                    out=wkr[:, 5 * kc * HEAD_DIM:5 * (kc + 1) * HEAD_DIM],
                    in_=wkr_d[:, 5 * kc * HEAD_DIM:5 * (kc + 1) * HEAD_DIM])
                nc.sync.dma_start(
                    out=wvr[:, 5 * kc * HEAD_DIM:5 * (kc + 1) * HEAD_DIM],
                    in_=wvr_d[:, 5 * kc * HEAD_DIM:5 * (kc + 1) * HEAD_DIM])
                nc.sync.dma_start(
                    out=xkvs[:].rearrange("p (k c) -> p k c", k=NW)[:, ks],
                    in_=xkv[:].rearrange("k p c -> p k c")[:, ks])
            nc.sync.dma_start(out=csk_s[:], in_=csk[:])
            nc.sync.dma_start(out=snk_s[:], in_=snk[:])

            wq_tiles = {}

            def load_w(src, m):
                t = wstr.tile([128, WIDTH], bf16, tag="w")
                nc.sync.dma_start(out=t[:], in_=src[m])
                wq_tiles[m] = t

            load_w(wq, 0)
            for kc in range(4):
                ks = slice(5 * kc, 5 * (kc + 1))
                nc.sync.dma_start(
                    out=xqs[:].rearrange("p (k c) -> p k c", k=NW)[:, ks],
                    in_=xq[:].rearrange("k p c -> p k c")[:, ks])
            nc.sync.dma_start(out=csq_s[:], in_=csq[:])
            nc.sync.dma_start(out=snq_s[:], in_=snq[:])
            load_w(wq, 1)

            # ones columns of V (denominator of softmax via matmul)
            nc.gpsimd.memset(
                vsb[:].rearrange("p (n v) -> p n v", n=NTT)[:, :, HEAD_DIM:VROW],
                1.0)

            def rope_evict(ps, cs, sn, dst0, dst1):
                """dst0 = ps0*cos - ps1*sin ; dst1 = ps1*cos + ps0*sin.

                ps: [128, n] PSUM fp32; cs/sn: [64, n] SBUF fp32 tables;
                dst0/dst1: bf16 SBUF APs [64, n] (head-dim halves)."""
                n = cs.shape[-1]
                t0 = tmpp.tile([64, QBLK], fp32, tag="t0", name="t0")
                t1 = tmpp.tile([64, QBLK], fp32, tag="t1", name="t1")
                nc.vector.tensor_mul(t0[:, :n], ps[0:64, :], cs)
                nc.vector.tensor_mul(t1[:, :n], ps[64:128, :], sn)
                nc.vector.tensor_sub(dst0, t0[:, :n], t1[:, :n])
                t2 = tmpp.tile([64, QBLK], fp32, tag="t0", name="t2")
                t3 = tmpp.tile([64, QBLK], fp32, tag="t1", name="t3")
                nc.vector.tensor_mul(t2[:, :n], ps[64:128, :], cs)
                nc.vector.tensor_mul(t3[:, :n], ps[0:64, :], sn)
                nc.vector.tensor_add(dst1, t2[:, :n], t3[:, :n])

            # ---- K/V shard projection (own 512 tokens) ----
            # kvsh cols: [0:512] rope'd K^T hd0:128, [512:1024] K^T hd128:256,
            # [1024:2048] V tiles (4 x [128tok, 256hd])
            psk0 = pp.tile([128, KVB], fp32, tag="pp", name="psk0")
            psk1 = pp.tile([128, KVB], fp32, tag="pp", name="psk1")
            psv = [op.tile([128, HEAD_DIM], fp32, tag="o", name=f"psv{mt}")
                   for mt in range(4)]
            for k in range(NW):
                nc.tensor.matmul(
                    psk0[:], lhsT=wkr[:, k * HEAD_DIM: k * HEAD_DIM + 128],
                    rhs=xkvs[:, k * KVB:(k + 1) * KVB],
                    start=(k == 0), stop=(k == NW - 1))
                nc.tensor.matmul(
                    psk1[:], lhsT=wkr[:, k * HEAD_DIM + 128:(k + 1) * HEAD_DIM],
                    rhs=xkvs[:, k * KVB:(k + 1) * KVB],
                    start=(k == 0), stop=(k == NW - 1))
                for mt in range(4):
                    nc.tensor.matmul(
                        psv[mt][:],
                        lhsT=xkvs[:, k * KVB + mt * 128: k * KVB + (mt + 1) * 128],
                        rhs=wvr[:, k * HEAD_DIM:(k + 1) * HEAD_DIM],
                        start=(k == 0), stop=(k == NW - 1))
            rope_evict(psk0, csk_s[:], snk_s[:],
                       kvsh[0:64, 0:KVB], kvsh[64:128, 0:KVB])
            nc.scalar.copy(out=kvsh[:, KVB:2 * KVB], in_=psk1[:])
            for mt in range(4):
                nc.scalar.copy(
                    out=kvsh[:, 1024 + mt * HEAD_DIM: 1024 + (mt + 1) * HEAD_DIM],
                    in_=psv[mt][:])

            # ---- K/V all-gather across the 4 cores of this batch ----
            nc.gpsimd.dma_start(out=kv_in[:], in_=kvsh[:])
            nc.gpsimd.collective_compute(
                "AllGather",
                mybir.AluOpType.bypass,
                replica_groups=[[0, 1, 2, 3], [4, 5, 6, 7]],
                ins=[kv_in.opt()],
                outs=[kv_out.opt()],
            )
            nc.gpsimd.dma_start(
                out=ktr0[:].rearrange("p (r c) -> p r c", r=4),
                in_=kv_out[:, :, 0:KVB].rearrange("r p c -> p r c"))
            nc.gpsimd.dma_start(
                out=ktr1[:].rearrange("p (r c) -> p r c", r=4),
                in_=kv_out[:, :, KVB:2 * KVB].rearrange("r p c -> p r c"))
            for r4 in range(4):
                nc.gpsimd.dma_start(
                    out=vsb[:].rearrange("p (n v) -> p n v", n=NTT)[
                        :, 4 * r4:4 * (r4 + 1), 0:HEAD_DIM],
                    in_=kv_out[r4, :, 1024:2048].rearrange(
                        "p (t v) -> p t v", t=4))

            # ---- Q projection -> rope'd Q^T stripes [qdim, QBLK] ----
            # stripe m: qdim rows [128m, 128m+128) = head m//2, half m%2
            for m in range(NW):
                if m + 2 < NW:
                    load_w(wq, m + 2)
                wq_m = wq_tiles.pop(m)
                ps = pp.tile([128, QBLK], fp32, tag="pp")
                for k in range(NW):
                    nc.tensor.matmul(
                        ps[:],
                        lhsT=wq_m[:, k * 128:(k + 1) * 128],
                        rhs=xqs[:, k * QBLK:(k + 1) * QBLK],
                        start=(k == 0),
                        stop=(k == NW - 1),
                    )
                dst = qtr[:, m * QBLK:(m + 1) * QBLK]
                if m % 2 == 0:  # rope half of the head dims
                    rope_evict(ps, csq_s[:], snq_s[:],
                               qtr[0:64, m * QBLK:(m + 1) * QBLK],
                               qtr[64:128, m * QBLK:(m + 1) * QBLK])
                else:           # passthrough half
                    nc.scalar.copy(out=dst, in_=ps[:])

            # masks + bias arrive behind the wq stripes, before attention
            nc.sync.dma_start(out=msk_s[:], in_=msk[:])
            nc.sync.dma_start(out=bia_s[:], in_=bia[:])

            # ---- attention (S^T layout: k on partitions, q on free dim) ----
            def evict_slot(h, p, o_tile):
                r = rcpp.tile([128, 1], fp32, tag="r")
                nc.vector.reciprocal(r[:], o_tile[:, HEAD_DIM:VROW])
                en = enp.tile([128, HEAD_DIM], bf16, tag="en")
                nc.scalar.activation(en[:], o_tile[:, 0:HEAD_DIM], Copy,
                                     scale=r[:])
                for hh in range(2):
                    tp = pp.tile([128, 128], bf16, tag="pp")
                    nc.tensor.matmul(
                        tp[:], lhsT=en[:, hh * 128:(hh + 1) * 128],
                        rhs=ident[:], is_transpose=True)
                    nc.vector.tensor_copy(
                        enct[:, (2 * h + hh) * QBLK + p * 128:
                             (2 * h + hh) * QBLK + (p + 1) * 128],
                        tp[:])

            # Software-pipelined over a flat (head, group) stream: the O
            # matmuls lag one group behind S/exp/mask so the Act+DVE latency
            # between S and O is never exposed on the tensor engine; the
            # eviction transposes lag one more group.
            all_groups = [(h, grp) for h in range(NUM_HEADS) for grp in TGROUPS]
            o_by_head = {}
            pts = {}

            def emit_s(i):
                h, grp = all_groups[i]
                if grp is TGROUPS[0]:
                    o_by_head[h] = [
                        op.tile([128, VROW], fp32, tag="o", name=f"o{h}_{p}")
                        for p in range(NSLOT)]
                nd = _needed(grp[0])
                gw = 128 * nd * len(grp)   # group column width
                st = stp.tile([128, QBLK], fp32, tag="st")
                for j, t in enumerate(grp):
                    cols = slice(j * 128 * nd, (j + 1) * 128 * nd)
                    nc.tensor.matmul(
                        st[:, cols], lhsT=ktr0[:, t * 128:(t + 1) * 128],
                        rhs=qtr[:, (2 * h) * QBLK:(2 * h) * QBLK + 128 * nd],
                        start=True, stop=False)
                    nc.tensor.matmul(
                        st[:, cols], lhsT=ktr1[:, t * 128:(t + 1) * 128],
                        rhs=qtr[:, (2 * h + 1) * QBLK:
                                 (2 * h + 1) * QBLK + 128 * nd],
                        start=False, stop=True)
                pt = ptp.tile([128, QBLK], bf16, tag="pt")
                # p = exp(s / sqrt(head_dim)), masked entries -> 0
                nc.scalar.activation(pt[:, :gw], st[:, :gw], Exp, scale=0.0625)
                nc.vector.tensor_mul(
                    pt[:, :gw], pt[:, :gw],
                    msk_s[:, MOFF[grp[0]]:MOFF[grp[0]] + gw])
                pts[i] = pt

            def emit_o(i):
                h, grp = all_groups[i]
                nd = _needed(grp[0])
                pt = pts.pop(i)
                for j, t in enumerate(grp):
                    for p in reversed(range(nd)):
                        nc.tensor.matmul(
                            o_by_head[h][p][:],
                            lhsT=pt[:, j * 128 * nd + p * 128:
                                    j * 128 * nd + (p + 1) * 128],
                            rhs=vsb[:, t * VROW:(t + 1) * VROW],
                            start=(t == 0),
                            stop=(t == NCOV[p] - 1),
                        )

            evq = []   # (h, p, en) awaiting their PE transposes

            def emit_evict_scale(i):
                h, grp = all_groups[i]
                for p in range(NSLOT):
                    if NCOV[p] - 1 != grp[-1]:
                        continue
                    o_tile = o_by_head[h][p]
                    r = rcpp.tile([128, 1], fp32, tag="r")
                    nc.vector.reciprocal(r[:], o_tile[:, HEAD_DIM:VROW])
                    en = enp.tile([128, HEAD_DIM], bf16, tag="en")
                    nc.scalar.activation(en[:], o_tile[:, 0:HEAD_DIM], Copy,
                                         scale=r[:])
                    evq.append((h, p, en))

            def emit_transposes():
                while evq:
                    h, p, en = evq.pop(0)
                    for hh in range(2):
                        tp = pp.tile([128, 128], bf16, tag="pp")
                        nc.tensor.matmul(
                            tp[:], lhsT=en[:, hh * 128:(hh + 1) * 128],
                            rhs=ident[:], is_transpose=True)
                        nc.vector.tensor_copy(
                            enct[:, (2 * h + hh) * QBLK + p * 128:
                                 (2 * h + hh) * QBLK + (p + 1) * 128],
                            tp[:])

            NG = len(all_groups)
            LAG = 2
            for i in range(NG + LAG):
                if i < NG:
                    emit_s(i)
                if i >= LAG:
                    emit_o(i - LAG)
                    emit_transposes()       # drain earlier evictions
                    emit_evict_scale(i - LAG)
            emit_transposes()

            # ---- final projection: out^T = wf @ enc^T + bias ----
            for m in range(NW):
                wf_m = wstr.tile([128, WIDTH], bf16, tag="w")
                nc.sync.dma_start(out=wf_m[:], in_=wf[m])
                ps = pp.tile([128, QBLK], fp32, tag="pp")
                if m < NW - 1:
                    for k in range(NW):
                        nc.tensor.matmul(
                            ps[:],
                            lhsT=wf_m[:, k * 128:(k + 1) * 128],
                            rhs=enct[:, k * QBLK:(k + 1) * QBLK],
                            start=(k == 0),
                            stop=(k == NW - 1),
                        )
                    osb = outp.tile([128, QBLK], fp32, tag="osb")
                    nc.vector.tensor_scalar_add(osb[:], ps[:], bia_s[:, m:m + 1])
                    nc.sync.dma_start(out=out[m], in_=osb[:])
                else:
                    # last stripe in column halves: the first half's
                    # bias/store drains under the second half's matmuls
                    osb = outp.tile([128, QBLK], fp32, tag="osb")
                    for j2 in range(2):
                        cj = slice(256 * j2, 256 * (j2 + 1))
                        for k in range(NW):
                            nc.tensor.matmul(
                                ps[:, cj],
                                lhsT=wf_m[:, k * 128:(k + 1) * 128],
                                rhs=enct[:, k * QBLK + 256 * j2:
                                         k * QBLK + 256 * (j2 + 1)],
                                start=(k == 0),
                                stop=(k == NW - 1),
                            )
                        nc.vector.tensor_scalar_add(
                            osb[:, cj], ps[:, cj], bia_s[:, m:m + 1])
                        nc.sync.dma_start(out=out[m][:, cj], in_=osb[:, cj])

    if not nc.is_finalized():
        nc.finalize()  # bacc register allocation — required before walrus compile
    return nc


def get_nc():
    if "nc" not in _NC_CACHE:
        _NC_CACHE["nc"] = _build_nc()
    return _NC_CACHE["nc"]


def _chunk_of_slot(r, p):
    """Physical 128-token query chunk held by rank r's slot p."""
    return r + 12 - 4 * p


def _host_prepare(x, segment_pos, wq, wk, wv, w_final, b_final):
    """Build shared + per-core device input arrays."""
    x = np.asarray(x, dtype=np.float32)
    segment_pos = np.asarray(segment_pos)
    wq = np.asarray(wq, dtype=np.float32)
    wk = np.asarray(wk, dtype=np.float32)
    wv = np.asarray(wv, dtype=np.float32)
    w_final = np.asarray(w_final, dtype=np.float32)
    b_final = np.asarray(b_final, dtype=np.float32)

    def stripes_sq(w):  # [WIDTH, WIDTH] torch-Linear weight -> [20,128,WIDTH] w^T stripes
        wt = np.ascontiguousarray(w.T)
        return np.ascontiguousarray(
            wt.reshape(NW, 128, NW, 128).transpose(2, 1, 0, 3).reshape(NW, 128, WIDTH)
        ).astype(BF16)

    def packed_kv(w):  # [HEAD_DIM, WIDTH] -> [128, NW*HEAD_DIM] w^T stripes
        return np.ascontiguousarray(
            w.T.reshape(NW, 128, HEAD_DIM).transpose(1, 0, 2).reshape(
                128, NW * HEAD_DIM)
        ).astype(BF16)

    shared = {
        "wq": stripes_sq(wq),
        "wf": stripes_sq(w_final),
        "wkr": packed_kv(wk),
        "wvr": packed_kv(wv),
        "bia": np.ascontiguousarray(b_final.reshape(NW, 128).T).astype(np.float32),
    }

    inv_freq = (
        1.0 / MAX_WAVELENGTH ** (2.0 * np.arange(HEAD_DIM // 4, dtype=np.float32)
                                 / (HEAD_DIM // 2))
    ).astype(np.float32)

    def cossin(pos):
        ang = inv_freq[:, None] * pos[None, :].astype(np.float32)
        return (np.cos(ang).astype(np.float32), np.sin(ang).astype(np.float32))

    in_maps = []
    for c in range(8):
        b, r = c // 4, c % 4
        pos = segment_pos[b].astype(np.float32)
        seg = np.cumsum((segment_pos[b] == 0).astype(np.int64))

        qidx = np.concatenate(
            [np.arange(128) + 128 * _chunk_of_slot(r, p) for p in range(NSLOT)])
        kidx = np.arange(KVB) + KVB * r

        xqc = np.ascontiguousarray(x[b][qidx].T).astype(BF16).reshape(NW, 128, QBLK)
        xkvc = np.ascontiguousarray(x[b][kidx].T).astype(BF16).reshape(NW, 128, KVB)

        csq_, snq_ = cossin(pos[qidx])
        csk_, snk_ = cossin(pos[kidx])

        # packed masks: for key tile t, needed(t) slot blocks of [128k,128q]
        mask = np.zeros((128, MCOLS), dtype=BF16)
        for t in range(NTT):
            tk = np.arange(128) + 128 * t
            for p in range(_needed(t)):
                tq = qidx[p * 128:(p + 1) * 128]
                allow = (
                    (tk[:, None] <= tq[None, :])
                    & (tq[None, :] <= tk[:, None] + WINDOW)
                    & (seg[tk][:, None] == seg[tq][None, :])
                )
                mask[:, MOFF[t] + 128 * p: MOFF[t] + 128 * (p + 1)] = allow
        in_maps.append(dict(
            shared,
            xq=xqc,
            xkv=xkvc,
            csk=csk_,
            snk=snk_,
            csq=csq_,
            snq=snq_,
            msk=mask,
        ))
    return in_maps


def _assemble(results):
    out = np.empty((B, T, WIDTH), dtype=np.float32)
    for c, res in enumerate(results):
        b, r = c // 4, c % 4
        o = np.asarray(res["out"], dtype=np.float32)  # [NW, 128, QBLK]
        for p in range(NSLOT):
            ch = _chunk_of_slot(r, p)
            out[b, 128 * ch:128 * (ch + 1), :] = (
                o[:, :, 128 * p:128 * (p + 1)].transpose(2, 0, 1).reshape(128, WIDTH)
            )
    return out


def kernel(x, segment_pos, wq, wk, wv, w_final, b_final):
    from concourse.bass_utils import run_bass_kernel_spmd

    nc = get_nc()
    in_maps = _host_prepare(x, segment_pos, wq, wk, wv, w_final, b_final)
    res = run_bass_kernel_spmd(nc, in_maps, list(range(8)))
    return _assemble(res.results)


# revision 23
# speedup vs baseline: 1.5646x; 1.0067x over previous
"""Trainium2 Bass kernel for a local-attention block (MQA, RoPE, causal mask).

Reference computation (B=2, T=2048, WIDTH=2560, 10 q-heads, 1 kv-head,
head_dim=256, window=2048 => mask reduces to causal & same-segment):

    q = x @ wq.T ; k = x @ wk.T ; v = x @ wv.T
    q, k = rope(q), rope(k)
    probs = softmax(q k^T / 16 + mask)
    out = (probs @ v) @ w_final.T + b_final

Sharding: 8 cores = 2 batches x 4 ranks. Load-balanced causal split: rank r
owns the four 128-token query chunks {r, r+4, r+8, r+12}, placed in "slots"
ordered by decreasing causal coverage (16, 12, 8, 4 key tiles of 128). For
key tile t only the first needed(t) = 4 - t//4 slots are processed, so the
S/O matmuls use contiguous column prefixes of the slot-ordered Q buffer and
every core does identical work (SPMD) with no fully-masked tiles computed.

K/V projection is sharded: each core projects+ropes K/V for its own 512
tokens, then an AllGather over the 4 cores of a batch reconstructs the full
K^T / V in every core. The gather runs on the collective cores concurrently
with the Q projection.

Device layouts are "feature on partitions, tokens on free dim" so every
matmul contraction maps to the 128-partition axis with no on-device
transposes (except 128x128 PE transposes of the attention output).
"""

import sys

import numpy as np

for _p in ("/opt/trn_rl_repo", "/root/.axon_site/_ro/trn_rl_repo"):
    if _p not in sys.path:
        sys.path.insert(0, _p)

import ml_dtypes

BF16 = ml_dtypes.bfloat16

B, T, WIDTH = 2, 2048, 2560
NUM_HEADS, HEAD_DIM = 10, 256
WINDOW = 2048
MAX_WAVELENGTH = 10000.0
QBLK = 512              # query tokens per core (4 chunks of 128, slot order)
KVB = 512               # kv tokens projected per core
NW = WIDTH // 128       # 20 width stripes
NTT = T // 128          # 16 key token tiles
VROW = HEAD_DIM + 1     # v columns + ones column (denominator trick)
NSLOT = 4
NCOV = [16, 12, 8, 4]   # key-tile coverage per slot
# key tiles grouped into shared PSUM banks (equal needed() within a group)
TGROUPS = [[0], [1], [2], [3], [4], [5], [6], [7], [8, 9], [10, 11],
           [12, 13, 14, 15]]


def _needed(t):
    return 4 - t // 4


# mask/pt column offset of key tile t
MOFF = [0] * NTT
for _t in range(1, NTT):
    MOFF[_t] = MOFF[_t - 1] + 128 * _needed(_t - 1)
MCOLS = MOFF[-1] + 128 * _needed(NTT - 1)  # 5120

_NC_CACHE = {}


def _build_nc():
    """Build the (single, SPMD-uniform) Bass/Tile program."""
    import concourse.bass as bass  # noqa: F401
    import concourse.mybir as mybir
    import concourse.tile as tile
    from concourse import bacc
    from concourse.masks import make_identity

    fp32 = mybir.dt.float32
    bf16 = mybir.dt.bfloat16
    Exp = mybir.ActivationFunctionType.Exp
    Copy = mybir.ActivationFunctionType.Copy

    nc = bacc.Bacc("TRN2", target_bir_lowering=False, debug=False, num_devices=8)

    # ---- DRAM I/O ----
    xq = nc.dram_tensor("xq", [NW, 128, QBLK], bf16, kind="ExternalInput")
    xkv = nc.dram_tensor("xkv", [NW, 128, KVB], bf16, kind="ExternalInput")
    wq = nc.dram_tensor("wq", [NW, 128, WIDTH], bf16, kind="ExternalInput")
    wkr_d = nc.dram_tensor("wkr", [128, NW * HEAD_DIM], bf16, kind="ExternalInput")
    wvr_d = nc.dram_tensor("wvr", [128, NW * HEAD_DIM], bf16, kind="ExternalInput")
    wf = nc.dram_tensor("wf", [NW, 128, WIDTH], bf16, kind="ExternalInput")
    csk = nc.dram_tensor("csk", [64, KVB], fp32, kind="ExternalInput")
    snk = nc.dram_tensor("snk", [64, KVB], fp32, kind="ExternalInput")
    csq = nc.dram_tensor("csq", [64, QBLK], fp32, kind="ExternalInput")
    snq = nc.dram_tensor("snq", [64, QBLK], fp32, kind="ExternalInput")
    msk = nc.dram_tensor("msk", [128, MCOLS], bf16, kind="ExternalInput")
    bia = nc.dram_tensor("bia", [128, NW], fp32, kind="ExternalInput")
    out = nc.dram_tensor("out", [NW, 128, QBLK], fp32, kind="ExternalOutput")

    with tile.TileContext(nc) as tc:
        with (
            tc.tile_pool(name="res", bufs=1) as res,
            tc.tile_pool(name="wstr", bufs=3) as wstr,
            tc.tile_pool(name="ptp", bufs=6) as ptp,
            tc.tile_pool(name="enp", bufs=4) as enp,
            tc.tile_pool(name="tmp", bufs=4) as tmpp,
            tc.tile_pool(name="rcp", bufs=4) as rcpp,
            tc.tile_pool(name="outp", bufs=3) as outp,
            tc.tile_pool(name="dram", bufs=1, space="DRAM") as dram,
            tc.tile_pool(name="pp", bufs=2, space="PSUM") as pp,
            tc.tile_pool(name="stp", bufs=2, space="PSUM") as stp,
            tc.tile_pool(name="op", bufs=4, space="PSUM") as op,
        ):
            # ---- resident SBUF tiles ----
            xkvs = res.tile([128, NW * KVB], bf16, tag="xkvs")
            xqs = res.tile([128, NW * QBLK], bf16, tag="xqs")
            qtr = res.tile([128, NW * QBLK], bf16, tag="qtr")   # rope'd Q^T
            ktr0 = res.tile([128, T], bf16, tag="ktr0")         # rope'd K^T hd 0:128
            ktr1 = res.tile([128, T], bf16, tag="ktr1")         # K^T hd 128:256
            vsb = res.tile([128, NTT * VROW], bf16, tag="vsb")  # V tiles + ones col
            kvsh = res.tile([128, 2048], bf16, tag="kvsh")      # own K/V shard
            wkr = res.tile([128, NW * HEAD_DIM], bf16, tag="wkr")
            wvr = res.tile([128, NW * HEAD_DIM], bf16, tag="wvr")
            enct = res.tile([128, NW * QBLK], bf16, tag="enct")
            csk_s = res.tile([64, KVB], fp32, tag="csk")
            snk_s = res.tile([64, KVB], fp32, tag="snk")
            csq_s = res.tile([64, QBLK], fp32, tag="csq")
            snq_s = res.tile([64, QBLK], fp32, tag="snq")
            msk_s = res.tile([128, MCOLS], bf16, tag="msk")
            bia_s = res.tile([128, NW], fp32, tag="bia")
            ident = res.tile([128, 128], bf16, tag="ident")

            make_identity(nc, ident[:])

            kv_in = dram.tile([128, 2048], bf16, tag="kvi")
            kv_out = dram.tile([4, 128, 2048], bf16, tag="kvo")

            # ---- input DMAs (SP queue, in consumption order) ----
            # wkr/wvr/xkv stream in interleaved chunks of width-stripes
            # (small first chunks) so the K/V projection starts early.
            kcs = [0, 2, 5, 10, 15, 20]
            for kc in range(len(kcs) - 1):
                ks = slice(kcs[kc], kcs[kc + 1])
                hs = slice(kcs[kc] * HEAD_DIM, kcs[kc + 1] * HEAD_DIM)
                nc.sync.dma_start(out=wkr[:, hs], in_=wkr_d[:, hs])
                nc.sync.dma_start(
                    out=xkvs[:].rearrange("p (k c) -> p k c", k=NW)[:, ks],
                    in_=xkv[:].rearrange("k p c -> p k c")[:, ks])
                nc.sync.dma_start(out=wvr[:, hs], in_=wvr_d[:, hs])
            nc.sync.dma_start(out=csk_s[:], in_=csk[:])
            nc.sync.dma_start(out=snk_s[:], in_=snk[:])

            wq_tiles = {}

            def load_w(src, m):
                t = wstr.tile([128, WIDTH], bf16, tag="w")
                nc.sync.dma_start(out=t[:], in_=src[m])
                wq_tiles[(src.name, m)] = t

            load_w(wq, 0)
            for kc in range(4):
                ks = slice(5 * kc, 5 * (kc + 1))
                nc.sync.dma_start(
                    out=xqs[:].rearrange("p (k c) -> p k c", k=NW)[:, ks],
                    in_=xq[:].rearrange("k p c -> p k c")[:, ks])
            nc.sync.dma_start(out=csq_s[:], in_=csq[:])
            nc.sync.dma_start(out=snq_s[:], in_=snq[:])
            load_w(wq, 1)

            # ones columns of V (denominator of softmax via matmul)
            nc.gpsimd.memset(
                vsb[:].rearrange("p (n v) -> p n v", n=NTT)[:, :, HEAD_DIM:VROW],
                1.0)

            def rope_evict(ps, cs, sn, dst0, dst1):
                """dst0 = ps0*cos - ps1*sin ; dst1 = ps1*cos + ps0*sin.

                ps: [128, n] PSUM fp32; cs/sn: [64, n] SBUF fp32 tables;
                dst0/dst1: bf16 SBUF APs [64, n] (head-dim halves)."""
                n = cs.shape[-1]
                t0 = tmpp.tile([64, QBLK], fp32, tag="t0", name="t0")
                t1 = tmpp.tile([64, QBLK], fp32, tag="t1", name="t1")
                nc.vector.tensor_mul(t0[:, :n], ps[0:64, :], cs)
                nc.vector.tensor_mul(t1[:, :n], ps[64:128, :], sn)
                nc.vector.tensor_sub(dst0, t0[:, :n], t1[:, :n])
                t2 = tmpp.tile([64, QBLK], fp32, tag="t0", name="t2")
                t3 = tmpp.tile([64, QBLK], fp32, tag="t1", name="t3")
                nc.vector.tensor_mul(t2[:, :n], ps[64:128, :], cs)
                nc.vector.tensor_mul(t3[:, :n], ps[0:64, :], sn)
                nc.vector.tensor_add(dst1, t2[:, :n], t3[:, :n])

            # ---- K/V shard projection (own 512 tokens) ----
            # kvsh cols: [0:512] rope'd K^T hd0:128, [512:1024] K^T hd128:256,
            # [1024:2048] V tiles (4 x [128tok, 256hd])
            psk0 = pp.tile([128, KVB], fp32, tag="pp", name="psk0")
            psk1 = pp.tile([128, KVB], fp32, tag="pp", name="psk1")
            psv = [op.tile([128, HEAD_DIM], fp32, tag="o", name=f"psv{mt}")
                   for mt in range(4)]
            for k in range(NW):
                nc.tensor.matmul(
                    psk0[:], lhsT=wkr[:, k * HEAD_DIM: k * HEAD_DIM + 128],
                    rhs=xkvs[:, k * KVB:(k + 1) * KVB],
                    start=(k == 0), stop=(k == NW - 1))
                nc.tensor.matmul(
                    psk1[:], lhsT=wkr[:, k * HEAD_DIM + 128:(k + 1) * HEAD_DIM],
                    rhs=xkvs[:, k * KVB:(k + 1) * KVB],
                    start=(k == 0), stop=(k == NW - 1))
                for mt in range(4):
                    nc.tensor.matmul(
                        psv[mt][:],
                        lhsT=xkvs[:, k * KVB + mt * 128: k * KVB + (mt + 1) * 128],
                        rhs=wvr[:, k * HEAD_DIM:(k + 1) * HEAD_DIM],
                        start=(k == 0), stop=(k == NW - 1))
            rope_evict(psk0, csk_s[:], snk_s[:],
                       kvsh[0:64, 0:KVB], kvsh[64:128, 0:KVB])
            nc.scalar.copy(out=kvsh[:, KVB:2 * KVB], in_=psk1[:])
            for mt in range(4):
                nc.scalar.copy(
                    out=kvsh[:, 1024 + mt * HEAD_DIM: 1024 + (mt + 1) * HEAD_DIM],
                    in_=psv[mt][:])

            # ---- K/V all-gather across the 4 cores of this batch ----
            nc.gpsimd.dma_start(out=kv_in[:], in_=kvsh[:])
            nc.gpsimd.collective_compute(
                "AllGather",
                mybir.AluOpType.bypass,
                replica_groups=[[0, 1, 2, 3], [4, 5, 6, 7]],
                ins=[kv_in.opt()],
                outs=[kv_out.opt()],
            )
            nc.gpsimd.dma_start(
                out=ktr0[:].rearrange("p (r c) -> p r c", r=4),
                in_=kv_out[:, :, 0:KVB].rearrange("r p c -> p r c"))
            nc.gpsimd.dma_start(
                out=ktr1[:].rearrange("p (r c) -> p r c", r=4),
                in_=kv_out[:, :, KVB:2 * KVB].rearrange("r p c -> p r c"))
            for r4 in range(4):
                nc.gpsimd.dma_start(
                    out=vsb[:].rearrange("p (n v) -> p n v", n=NTT)[
                        :, 4 * r4:4 * (r4 + 1), 0:HEAD_DIM],
                    in_=kv_out[r4, :, 1024:2048].rearrange(
                        "p (t v) -> p t v", t=4))

            # ---- Q projection -> rope'd Q^T stripes [qdim, QBLK] ----
            # stripe m: qdim rows [128m, 128m+128) = head m//2, half m%2
            for m in range(NW):
                if m + 2 < NW:
                    load_w(wq, m + 2)
                wq_m = wq_tiles.pop(("wq", m))
                ps = pp.tile([128, QBLK], fp32, tag="pp")
                for k in range(NW):
                    nc.tensor.matmul(
                        ps[:],
                        lhsT=wq_m[:, k * 128:(k + 1) * 128],
                        rhs=xqs[:, k * QBLK:(k + 1) * QBLK],
                        start=(k == 0),
                        stop=(k == NW - 1),
                    )
                dst = qtr[:, m * QBLK:(m + 1) * QBLK]
                if m % 2 == 0:  # rope half of the head dims
                    rope_evict(ps, csq_s[:], snq_s[:],
                               qtr[0:64, m * QBLK:(m + 1) * QBLK],
                               qtr[64:128, m * QBLK:(m + 1) * QBLK])
                else:           # passthrough half
                    nc.scalar.copy(out=dst, in_=ps[:])

            # masks + bias arrive behind the wq stripes, before attention
            nc.sync.dma_start(out=msk_s[:], in_=msk[:])
            nc.sync.dma_start(out=bia_s[:], in_=bia[:])

            # ---- attention (S^T layout: k on partitions, q on free dim) ----
            def evict_slot(h, p, o_tile):
                r = rcpp.tile([128, 1], fp32, tag="r")
                nc.vector.reciprocal(r[:], o_tile[:, HEAD_DIM:VROW])
                en = enp.tile([128, HEAD_DIM], bf16, tag="en")
                nc.scalar.activation(en[:], o_tile[:, 0:HEAD_DIM], Copy,
                                     scale=r[:])
                for hh in range(2):
                    tp = pp.tile([128, 128], bf16, tag="pp")
                    nc.tensor.matmul(
                        tp[:], lhsT=en[:, hh * 128:(hh + 1) * 128],
                        rhs=ident[:], is_transpose=True)
                    nc.vector.tensor_copy(
                        enct[:, (2 * h + hh) * QBLK + p * 128:
                             (2 * h + hh) * QBLK + (p + 1) * 128],
                        tp[:])

            # Software-pipelined over a flat (head, group) stream: the O
            # matmuls lag one group behind S/exp/mask so the Act+DVE latency
            # between S and O is never exposed on the tensor engine; the
            # eviction transposes lag one more group.
            all_groups = [(h, grp) for h in range(NUM_HEADS) for grp in TGROUPS]
            o_by_head = {}
            pts = {}

            def emit_s(i):
                h, grp = all_groups[i]
                if grp is TGROUPS[0]:
                    o_by_head[h] = [
                        op.tile([128, VROW], fp32, tag="o", name=f"o{h}_{p}")
                        for p in range(NSLOT)]
                nd = _needed(grp[0])
                gw = 128 * nd * len(grp)   # group column width
                st = stp.tile([128, QBLK], fp32, tag="st")
                for j, t in enumerate(grp):
                    cols = slice(j * 128 * nd, (j + 1) * 128 * nd)
                    nc.tensor.matmul(
                        st[:, cols], lhsT=ktr0[:, t * 128:(t + 1) * 128],
                        rhs=qtr[:, (2 * h) * QBLK:(2 * h) * QBLK + 128 * nd],
                        start=True, stop=False)
                    nc.tensor.matmul(
                        st[:, cols], lhsT=ktr1[:, t * 128:(t + 1) * 128],
                        rhs=qtr[:, (2 * h + 1) * QBLK:
                                 (2 * h + 1) * QBLK + 128 * nd],
                        start=False, stop=True)
                pt = ptp.tile([128, QBLK], bf16, tag="pt")
                # p = exp(s / sqrt(head_dim)), masked entries -> 0
                nc.scalar.activation(pt[:, :gw], st[:, :gw], Exp, scale=0.0625)
                nc.vector.tensor_mul(
                    pt[:, :gw], pt[:, :gw],
                    msk_s[:, MOFF[grp[0]]:MOFF[grp[0]] + gw])
                pts[i] = pt

            def emit_o(i):
                h, grp = all_groups[i]
                nd = _needed(grp[0])
                pt = pts.pop(i)
                for j, t in enumerate(grp):
                    for p in reversed(range(nd)):
                        nc.tensor.matmul(
                            o_by_head[h][p][:],
                            lhsT=pt[:, j * 128 * nd + p * 128:
                                    j * 128 * nd + (p + 1) * 128],
                            rhs=vsb[:, t * VROW:(t + 1) * VROW],
                            start=(t == 0),
                            stop=(t == NCOV[p] - 1),
                        )

            evq = []   # (h, p, en) awaiting their PE transposes

            def emit_evict_scale(i):
                h, grp = all_groups[i]
                for p in range(NSLOT):
                    if NCOV[p] - 1 != grp[-1]:
                        continue
                    o_tile = o_by_head[h][p]
                    r = rcpp.tile([128, 1], fp32, tag="r")
                    nc.vector.reciprocal(r[:], o_tile[:, HEAD_DIM:VROW])
                    en = enp.tile([128, HEAD_DIM], bf16, tag="en")
                    nc.scalar.activation(en[:], o_tile[:, 0:HEAD_DIM], Copy,
                                         scale=r[:])
                    evq.append((h, p, en))

            def emit_transposes():
                while evq:
                    h, p, en = evq.pop(0)
                    for hh in range(2):
                        tp = pp.tile([128, 128], bf16, tag="pp")
                        nc.tensor.matmul(
                            tp[:], lhsT=en[:, hh * 128:(hh + 1) * 128],
                            rhs=ident[:], is_transpose=True)
                        nc.vector.tensor_copy(
                            enct[:, (2 * h + hh) * QBLK + p * 128:
                                 (2 * h + hh) * QBLK + (p + 1) * 128],
                            tp[:])

            # wf prefetch: triggers fire on the idle SP queue during attention
            load_w(wf, 0)
            load_w(wf, 1)

            NG = len(all_groups)
            LAG = 2
            for i in range(NG):
                emit_s(i)
                if i >= LAG:
                    emit_o(i - LAG)
                    emit_transposes()       # drain earlier evictions
                    emit_evict_scale(i - LAG)

            # Attention tail interleaved with the first final-proj stripes:
            # heads 0..8 (k=0..17) of stripes 0/1 accumulate while head 9's
            # last O/eviction chains drain, hiding their latency.
            fin_ps = {}

            def final_partial(m, kr):
                if m not in fin_ps:
                    fin_ps[m] = stp.tile([128, QBLK], fp32, tag="st",
                                         name=f"fps{m}")
                for k in kr:
                    nc.tensor.matmul(
                        fin_ps[m][:],
                        lhsT=wq_tiles[("wf", m)][:, k * 128:(k + 1) * 128],
                        rhs=enct[:, k * QBLK:(k + 1) * QBLK],
                        start=(k == 0),
                        stop=(k == NW - 1),
                    )

            emit_o(NG - 2)
            emit_evict_scale(NG - 2)
            final_partial(0, range(0, 18))
            emit_o(NG - 1)
            emit_evict_scale(NG - 1)
            final_partial(1, range(0, 18))
            emit_transposes()

            # ---- final projection: out^T = wf @ enc^T + bias ----
            for m in range(NW):
                if m + 2 < NW:
                    load_w(wf, m + 2)
                wf_m = wq_tiles.pop(("wf", m))
                if m in fin_ps:  # heads 0..8 already accumulated above
                    ps = fin_ps.pop(m)
                    for k in range(18, NW):
                        nc.tensor.matmul(
                            ps[:],
                            lhsT=wf_m[:, k * 128:(k + 1) * 128],
                            rhs=enct[:, k * QBLK:(k + 1) * QBLK],
                            start=(k == 0),
                            stop=(k == NW - 1),
                        )
                    osb = outp.tile([128, QBLK], fp32, tag="osb")
                    nc.vector.tensor_scalar_add(osb[:], ps[:], bia_s[:, m:m + 1])
                    nc.sync.dma_start(out=out[m], in_=osb[:])
                    continue
                ps = pp.tile([128, QBLK], fp32, tag="pp")
                if m < NW - 1:
                    for k in range(NW):
                        nc.tensor.matmul(
                            ps[:],
                            lhsT=wf_m[:, k * 128:(k + 1) * 128],
                            rhs=enct[:, k * QBLK:(k + 1) * QBLK],
                            start=(k == 0),
                            stop=(k == NW - 1),
                        )
                    osb = outp.tile([128, QBLK], fp32, tag="osb")
                    nc.vector.tensor_scalar_add(osb[:], ps[:], bia_s[:, m:m + 1])
                    nc.sync.dma_start(out=out[m], in_=osb[:])
                else:
                    # last stripe in column halves: the first half's
                    # bias/store drains under the second half's matmuls
                    osb = outp.tile([128, QBLK], fp32, tag="osb")
                    for j2 in range(2):
                        cj = slice(256 * j2, 256 * (j2 + 1))
                        for k in range(NW):
                            nc.tensor.matmul(
                                ps[:, cj],
                                lhsT=wf_m[:, k * 128:(k + 1) * 128],
                                rhs=enct[:, k * QBLK + 256 * j2:
                                         k * QBLK + 256 * (j2 + 1)],
                                start=(k == 0),
                                stop=(k == NW - 1),
                            )
                        nc.vector.tensor_scalar_add(
                            osb[:, cj], ps[:, cj], bia_s[:, m:m + 1])
                        nc.sync.dma_start(out=out[m][:, cj], in_=osb[:, cj])

    if not nc.is_finalized():
        nc.finalize()  # bacc register allocation — required before walrus compile
    return nc


def get_nc():
    if "nc" not in _NC_CACHE:
        _NC_CACHE["nc"] = _build_nc()
    return _NC_CACHE["nc"]


def _chunk_of_slot(r, p):
    """Physical 128-token query chunk held by rank r's slot p."""
    return r + 12 - 4 * p


def _host_prepare(x, segment_pos, wq, wk, wv, w_final, b_final):
    """Build shared + per-core device input arrays."""
    x = np.asarray(x, dtype=np.float32)
    segment_pos = np.asarray(segment_pos)
    wq = np.asarray(wq, dtype=np.float32)
    wk = np.asarray(wk, dtype=np.float32)
    wv = np.asarray(wv, dtype=np.float32)
    w_final = np.asarray(w_final, dtype=np.float32)
    b_final = np.asarray(b_final, dtype=np.float32)

    def stripes_sq(w):  # [WIDTH, WIDTH] torch-Linear weight -> [20,128,WIDTH] w^T stripes
        wt = np.ascontiguousarray(w.T)
        return np.ascontiguousarray(
            wt.reshape(NW, 128, NW, 128).transpose(2, 1, 0, 3).reshape(NW, 128, WIDTH)
        ).astype(BF16)

    def packed_kv(w):  # [HEAD_DIM, WIDTH] -> [128, NW*HEAD_DIM] w^T stripes
        return np.ascontiguousarray(
            w.T.reshape(NW, 128, HEAD_DIM).transpose(1, 0, 2).reshape(
                128, NW * HEAD_DIM)
        ).astype(BF16)

    shared = {
        "wq": stripes_sq(wq),
        "wf": stripes_sq(w_final),
        "wkr": packed_kv(wk),
        "wvr": packed_kv(wv),
        "bia": np.ascontiguousarray(b_final.reshape(NW, 128).T).astype(np.float32),
    }

    inv_freq = (
        1.0 / MAX_WAVELENGTH ** (2.0 * np.arange(HEAD_DIM // 4, dtype=np.float32)
                                 / (HEAD_DIM // 2))
    ).astype(np.float32)

    def cossin(pos):
        ang = inv_freq[:, None] * pos[None, :].astype(np.float32)
        return (np.cos(ang).astype(np.float32), np.sin(ang).astype(np.float32))

    in_maps = []
    for c in range(8):
        b, r = c // 4, c % 4
        pos = segment_pos[b].astype(np.float32)
        seg = np.cumsum((segment_pos[b] == 0).astype(np.int64))

        qidx = np.concatenate(
            [np.arange(128) + 128 * _chunk_of_slot(r, p) for p in range(NSLOT)])
        kidx = np.arange(KVB) + KVB * r

        xqc = np.ascontiguousarray(x[b][qidx].T).astype(BF16).reshape(NW, 128, QBLK)
        xkvc = np.ascontiguousarray(x[b][kidx].T).astype(BF16).reshape(NW, 128, KVB)

        csq_, snq_ = cossin(pos[qidx])
        csk_, snk_ = cossin(pos[kidx])

        # packed masks: for key tile t, needed(t) slot blocks of [128k,128q]
        mask = np.zeros((128, MCOLS), dtype=BF16)
        for t in range(NTT):
            tk = np.arange(128) + 128 * t
            for p in range(_needed(t)):
                tq = qidx[p * 128:(p + 1) * 128]
                allow = (
                    (tk[:, None] <= tq[None, :])
                    & (tq[None, :] <= tk[:, None] + WINDOW)
                    & (seg[tk][:, None] == seg[tq][None, :])
                )
                mask[:, MOFF[t] + 128 * p: MOFF[t] + 128 * (p + 1)] = allow
        in_maps.append(dict(
            shared,
            xq=xqc,
            xkv=xkvc,
            csk=csk_,
            snk=snk_,
            csq=csq_,
            snq=snq_,
            msk=mask,
        ))
    return in_maps


def _assemble(results):
    out = np.empty((B, T, WIDTH), dtype=np.float32)
    for c, res in enumerate(results):
        b, r = c // 4, c % 4
        o = np.asarray(res["out"], dtype=np.float32)  # [NW, 128, QBLK]
        for p in range(NSLOT):
            ch = _chunk_of_slot(r, p)
            out[b, 128 * ch:128 * (ch + 1), :] = (
                o[:, :, 128 * p:128 * (p + 1)].transpose(2, 0, 1).reshape(128, WIDTH)
            )
    return out


def kernel(x, segment_pos, wq, wk, wv, w_final, b_final):
    from concourse.bass_utils import run_bass_kernel_spmd

    nc = get_nc()
    in_maps = _host_prepare(x, segment_pos, wq, wk, wv, w_final, b_final)
    res = run_bass_kernel_spmd(nc, in_maps, list(range(8)))
    return _assemble(res.results)


# revision 29
# speedup vs baseline: 1.5768x; 1.0078x over previous
"""Trainium2 Bass kernel for a local-attention block (MQA, RoPE, causal mask).

Reference computation (B=2, T=2048, WIDTH=2560, 10 q-heads, 1 kv-head,
head_dim=256, window=2048 => mask reduces to causal & same-segment):

    q = x @ wq.T ; k = x @ wk.T ; v = x @ wv.T
    q, k = rope(q), rope(k)
    probs = softmax(q k^T / 16 + mask)
    out = (probs @ v) @ w_final.T + b_final

Sharding: 8 cores = 2 batches x 4 ranks. Load-balanced causal split: rank r
owns the four 128-token query chunks {r, r+4, r+8, r+12}, placed in "slots"
ordered by decreasing causal coverage (16, 12, 8, 4 key tiles of 128). For
key tile t only the first needed(t) = 4 - t//4 slots are processed, so the
S/O matmuls use contiguous column prefixes of the slot-ordered Q buffer and
every core does identical work (SPMD) with no fully-masked tiles computed.

K/V projection is sharded: each core projects+ropes K/V for its own 512
tokens, then an AllGather over the 4 cores of a batch reconstructs the full
K^T / V in every core. The gather runs on the collective cores concurrently
with the Q projection.

Device layouts are "feature on partitions, tokens on free dim" so every
matmul contraction maps to the 128-partition axis with no on-device
transposes (except 128x128 PE transposes of the attention output).
"""

import sys

import numpy as np

for _p in ("/opt/trn_rl_repo", "/root/.axon_site/_ro/trn_rl_repo"):
    if _p not in sys.path:
        sys.path.insert(0, _p)

import ml_dtypes

BF16 = ml_dtypes.bfloat16

B, T, WIDTH = 2, 2048, 2560
NUM_HEADS, HEAD_DIM = 10, 256
WINDOW = 2048
MAX_WAVELENGTH = 10000.0
QBLK = 512              # query tokens per core (4 chunks of 128, slot order)
KVB = 512               # kv tokens projected per core
NW = WIDTH // 128       # 20 width stripes
NTT = T // 128          # 16 key token tiles
VROW = HEAD_DIM + 1     # v columns + ones column (denominator trick)
NSLOT = 4
NCOV = [16, 12, 8, 4]   # key-tile coverage per slot
# key tiles grouped into shared PSUM banks (equal needed() within a group)
TGROUPS = [[0], [1], [2], [3], [4], [5], [6], [7], [8, 9], [10, 11],
           [12, 13, 14, 15]]


def _needed(t):
    return 4 - t // 4


# mask/pt column offset of key tile t
MOFF = [0] * NTT
for _t in range(1, NTT):
    MOFF[_t] = MOFF[_t - 1] + 128 * _needed(_t - 1)
MCOLS = MOFF[-1] + 128 * _needed(NTT - 1)  # 5120

_NC_CACHE = {}


def _build_nc():
    """Build the (single, SPMD-uniform) Bass/Tile program."""
    import concourse.bass as bass  # noqa: F401
    import concourse.mybir as mybir
    import concourse.tile as tile
    from concourse import bacc
    from concourse.masks import make_identity

    fp32 = mybir.dt.float32
    bf16 = mybir.dt.bfloat16
    Exp = mybir.ActivationFunctionType.Exp
    Copy = mybir.ActivationFunctionType.Copy

    nc = bacc.Bacc("TRN2", target_bir_lowering=False, debug=False, num_devices=8)

    # ---- DRAM I/O ----
    xq = nc.dram_tensor("xq", [NW, 128, QBLK], bf16, kind="ExternalInput")
    xkv = nc.dram_tensor("xkv", [NW, 128, KVB], bf16, kind="ExternalInput")
    wq = nc.dram_tensor("wq", [NW, 128, WIDTH], bf16, kind="ExternalInput")
    wkr_d = nc.dram_tensor("wkr", [128, NW * HEAD_DIM], bf16, kind="ExternalInput")
    wvr_d = nc.dram_tensor("wvr", [128, NW * HEAD_DIM], bf16, kind="ExternalInput")
    wf = nc.dram_tensor("wf", [NW, 128, WIDTH], bf16, kind="ExternalInput")
    csk = nc.dram_tensor("csk", [64, KVB], fp32, kind="ExternalInput")
    snk = nc.dram_tensor("snk", [64, KVB], fp32, kind="ExternalInput")
    csq = nc.dram_tensor("csq", [64, QBLK], fp32, kind="ExternalInput")
    snq = nc.dram_tensor("snq", [64, QBLK], fp32, kind="ExternalInput")
    msk = nc.dram_tensor("msk", [128, MCOLS], bf16, kind="ExternalInput")
    bia = nc.dram_tensor("bia", [128, NW], fp32, kind="ExternalInput")
    out = nc.dram_tensor("out", [NW, 128, QBLK], fp32, kind="ExternalOutput")

    with tile.TileContext(nc) as tc:
        with (
            tc.tile_pool(name="res", bufs=1) as res,
            tc.tile_pool(name="wstr", bufs=4) as wstr,
            tc.tile_pool(name="ptp", bufs=6) as ptp,
            tc.tile_pool(name="enp", bufs=4) as enp,
            tc.tile_pool(name="tmp", bufs=4) as tmpp,
            tc.tile_pool(name="rcp", bufs=4) as rcpp,
            tc.tile_pool(name="outp", bufs=3) as outp,
            tc.tile_pool(name="dram", bufs=1, space="DRAM") as dram,
            tc.tile_pool(name="pp", bufs=2, space="PSUM") as pp,
            tc.tile_pool(name="stp", bufs=2, space="PSUM") as stp,
            tc.tile_pool(name="op", bufs=4, space="PSUM") as op,
        ):
            # ---- resident SBUF tiles ----
            xkvs = res.tile([128, NW * KVB], bf16, tag="xkvs")
            xqs = res.tile([128, NW * QBLK], bf16, tag="xqs")
            qtr = res.tile([128, NW * QBLK], bf16, tag="qtr")   # rope'd Q^T
            ktr0 = res.tile([128, T], bf16, tag="ktr0")         # rope'd K^T hd 0:128
            ktr1 = res.tile([128, T], bf16, tag="ktr1")         # K^T hd 128:256
            vsb = res.tile([128, NTT * VROW], bf16, tag="vsb")  # V tiles + ones col
            kvsh = res.tile([128, 2048], bf16, tag="kvsh")      # own K/V shard
            wkr = res.tile([128, NW * HEAD_DIM], bf16, tag="wkr")
            wvr = res.tile([128, NW * HEAD_DIM], bf16, tag="wvr")
            enct = res.tile([128, NW * QBLK], bf16, tag="enct")
            csk_s = res.tile([64, KVB], fp32, tag="csk")
            snk_s = res.tile([64, KVB], fp32, tag="snk")
            csq_s = res.tile([64, QBLK], fp32, tag="csq")
            snq_s = res.tile([64, QBLK], fp32, tag="snq")
            msk_s = res.tile([128, MCOLS], bf16, tag="msk")
            bia_s = res.tile([128, NW], fp32, tag="bia")
            kv_in = dram.tile([128, 2048], bf16, tag="kvi")
            kv_out = dram.tile([4, 128, 2048], bf16, tag="kvo")

            # ---- input DMAs (SP queue, in consumption order) ----
            # wkr/wvr/xkv stream in interleaved chunks of width-stripes
            # (small first chunks) so the K/V projection starts early.
            kcs = [0, 2, 5, 10, 15, 20]
            for kc in range(len(kcs) - 1):
                ks = slice(kcs[kc], kcs[kc + 1])
                hs = slice(kcs[kc] * HEAD_DIM, kcs[kc + 1] * HEAD_DIM)
                nc.sync.dma_start(out=wkr[:, hs], in_=wkr_d[:, hs])
                nc.sync.dma_start(
                    out=xkvs[:].rearrange("p (k c) -> p k c", k=NW)[:, ks],
                    in_=xkv[:].rearrange("k p c -> p k c")[:, ks])
                nc.sync.dma_start(out=wvr[:, hs], in_=wvr_d[:, hs])
            nc.sync.dma_start(out=csk_s[:], in_=csk[:])
            nc.sync.dma_start(out=snk_s[:], in_=snk[:])

            wq_tiles = {}

            def load_w(src, m):
                t = wstr.tile([128, WIDTH], bf16, tag="w")
                nc.sync.dma_start(out=t[:], in_=src[m])
                wq_tiles[(src.name, m)] = t

            load_w(wq, 0)
            load_w(wq, 1)
            load_w(wq, 2)
            for kc in range(4):
                ks = slice(5 * kc, 5 * (kc + 1))
                nc.sync.dma_start(
                    out=xqs[:].rearrange("p (k c) -> p k c", k=NW)[:, ks],
                    in_=xq[:].rearrange("k p c -> p k c")[:, ks])
            nc.sync.dma_start(out=csq_s[:], in_=csq[:])
            nc.sync.dma_start(out=snq_s[:], in_=snq[:])

            # ones columns of V (denominator of softmax via matmul)
            nc.gpsimd.memset(
                vsb[:].rearrange("p (n v) -> p n v", n=NTT)[:, :, HEAD_DIM:VROW],
                1.0)

            def rope_evict(ps, cs, sn, dst0, dst1):
                """dst0 = ps0*cos - ps1*sin ; dst1 = ps1*cos + ps0*sin.

                ps: [128, n] PSUM fp32; cs/sn: [64, n] SBUF fp32 tables;
                dst0/dst1: bf16 SBUF APs [64, n] (head-dim halves)."""
                n = cs.shape[-1]
                t0 = tmpp.tile([64, QBLK], fp32, tag="t0", name="t0")
                t1 = tmpp.tile([64, QBLK], fp32, tag="t1", name="t1")
                nc.vector.tensor_mul(t0[:, :n], ps[0:64, :], cs)
                nc.vector.tensor_mul(t1[:, :n], ps[64:128, :], sn)
                nc.vector.tensor_sub(dst0, t0[:, :n], t1[:, :n])
                t2 = tmpp.tile([64, QBLK], fp32, tag="t0", name="t2")
                t3 = tmpp.tile([64, QBLK], fp32, tag="t1", name="t3")
                nc.vector.tensor_mul(t2[:, :n], ps[64:128, :], cs)
                nc.vector.tensor_mul(t3[:, :n], ps[0:64, :], sn)
                nc.vector.tensor_add(dst1, t2[:, :n], t3[:, :n])

            # ---- K/V shard projection (own 512 tokens) ----
            # kvsh cols: [0:512] rope'd K^T hd0:128, [512:1024] K^T hd128:256,
            # [1024:2048] V tiles (4 x [128tok, 256hd])
            psk0 = pp.tile([128, KVB], fp32, tag="pp", name="psk0")
            psk1 = pp.tile([128, KVB], fp32, tag="pp", name="psk1")
            psv = [op.tile([128, HEAD_DIM], fp32, tag="o", name=f"psv{mt}")
                   for mt in range(4)]
            for k in range(NW):
                nc.tensor.matmul(
                    psk0[:], lhsT=wkr[:, k * HEAD_DIM: k * HEAD_DIM + 128],
                    rhs=xkvs[:, k * KVB:(k + 1) * KVB],
                    start=(k == 0), stop=(k == NW - 1))
                nc.tensor.matmul(
                    psk1[:], lhsT=wkr[:, k * HEAD_DIM + 128:(k + 1) * HEAD_DIM],
                    rhs=xkvs[:, k * KVB:(k + 1) * KVB],
                    start=(k == 0), stop=(k == NW - 1))
                for mt in range(4):
                    nc.tensor.matmul(
                        psv[mt][:],
                        lhsT=xkvs[:, k * KVB + mt * 128: k * KVB + (mt + 1) * 128],
                        rhs=wvr[:, k * HEAD_DIM:(k + 1) * HEAD_DIM],
                        start=(k == 0), stop=(k == NW - 1))
            rope_evict(psk0, csk_s[:], snk_s[:],
                       kvsh[0:64, 0:KVB], kvsh[64:128, 0:KVB])
            nc.scalar.copy(out=kvsh[:, KVB:2 * KVB], in_=psk1[:])
            for mt in range(4):
                nc.scalar.copy(
                    out=kvsh[:, 1024 + mt * HEAD_DIM: 1024 + (mt + 1) * HEAD_DIM],
                    in_=psv[mt][:])

            # ---- K/V all-gather across the 4 cores of this batch ----
            nc.gpsimd.dma_start(out=kv_in[:], in_=kvsh[:])
            nc.gpsimd.collective_compute(
                "AllGather",
                mybir.AluOpType.bypass,
                replica_groups=[[0, 1, 2, 3], [4, 5, 6, 7]],
                ins=[kv_in.opt()],
                outs=[kv_out.opt()],
            )
            nc.gpsimd.dma_start(
                out=ktr0[:].rearrange("p (r c) -> p r c", r=4),
                in_=kv_out[:, :, 0:KVB].rearrange("r p c -> p r c"))
            nc.gpsimd.dma_start(
                out=ktr1[:].rearrange("p (r c) -> p r c", r=4),
                in_=kv_out[:, :, KVB:2 * KVB].rearrange("r p c -> p r c"))
            for r4 in range(4):
                nc.gpsimd.dma_start(
                    out=vsb[:].rearrange("p (n v) -> p n v", n=NTT)[
                        :, 4 * r4:4 * (r4 + 1), 0:HEAD_DIM],
                    in_=kv_out[r4, :, 1024:2048].rearrange(
                        "p (t v) -> p t v", t=4))

            # ---- Q projection -> rope'd Q^T stripes [qdim, QBLK] ----
            # stripe m: qdim rows [128m, 128m+128) = head m//2, half m%2
            for m in range(NW):
                if m + 3 < NW:
                    load_w(wq, m + 3)
                wq_m = wq_tiles.pop(("wq", m))
                ps = pp.tile([128, QBLK], fp32, tag="pp")
                for k in range(NW):
                    nc.tensor.matmul(
                        ps[:],
                        lhsT=wq_m[:, k * 128:(k + 1) * 128],
                        rhs=xqs[:, k * QBLK:(k + 1) * QBLK],
                        start=(k == 0),
                        stop=(k == NW - 1),
                    )
                dst = qtr[:, m * QBLK:(m + 1) * QBLK]
                if m % 2 == 0:  # rope half of the head dims
                    rope_evict(ps, csq_s[:], snq_s[:],
                               qtr[0:64, m * QBLK:(m + 1) * QBLK],
                               qtr[64:128, m * QBLK:(m + 1) * QBLK])
                else:           # passthrough half
                    nc.scalar.copy(out=dst, in_=ps[:])

            # masks + bias arrive behind the wq stripes, before attention
            nc.sync.dma_start(out=msk_s[:], in_=msk[:])
            nc.sync.dma_start(out=bia_s[:], in_=bia[:])

            # ---- attention (S^T layout: k on partitions, q on free dim) ----
            def evict_slot(h, p, o_tile):
                r = rcpp.tile([128, 1], fp32, tag="r")
                nc.vector.reciprocal(r[:], o_tile[:, HEAD_DIM:VROW])
                en = enp.tile([128, HEAD_DIM], bf16, tag="en")
                nc.scalar.activation(en[:], o_tile[:, 0:HEAD_DIM], Copy,
                                     scale=r[:])
                for hh in range(2):
                    tp = pp.tile([128, 128], bf16, tag="pp")
                    nc.tensor.matmul(
                        tp[:], lhsT=en[:, hh * 128:(hh + 1) * 128],
                        rhs=ident[:], is_transpose=True)
                    nc.vector.tensor_copy(
                        enct[:, (2 * h + hh) * QBLK + p * 128:
                             (2 * h + hh) * QBLK + (p + 1) * 128],
                        tp[:])

            # Software-pipelined over a flat (head, group) stream: the O
            # matmuls lag one group behind S/exp/mask so the Act+DVE latency
            # between S and O is never exposed on the tensor engine; the
            # eviction transposes lag one more group.
            all_groups = [(h, grp) for h in range(NUM_HEADS) for grp in TGROUPS]
            o_by_head = {}
            pts = {}

            def emit_s(i):
                h, grp = all_groups[i]
                if grp is TGROUPS[0]:
                    o_by_head[h] = [
                        op.tile([128, VROW], fp32, tag="o", name=f"o{h}_{p}")
                        for p in range(NSLOT)]
                nd = _needed(grp[0])
                gw = 128 * nd * len(grp)   # group column width
                st = stp.tile([128, QBLK], fp32, tag="st")
                for j, t in enumerate(grp):
                    cols = slice(j * 128 * nd, (j + 1) * 128 * nd)
                    nc.tensor.matmul(
                        st[:, cols], lhsT=ktr0[:, t * 128:(t + 1) * 128],
                        rhs=qtr[:, (2 * h) * QBLK:(2 * h) * QBLK + 128 * nd],
                        start=True, stop=False)
                    nc.tensor.matmul(
                        st[:, cols], lhsT=ktr1[:, t * 128:(t + 1) * 128],
                        rhs=qtr[:, (2 * h + 1) * QBLK:
                                 (2 * h + 1) * QBLK + 128 * nd],
                        start=False, stop=True)
                pt = ptp.tile([128, QBLK], bf16, tag="pt")
                # p = exp(s / sqrt(head_dim)), masked entries -> 0
                nc.scalar.activation(pt[:, :gw], st[:, :gw], Exp, scale=0.0625)
                nc.vector.tensor_mul(
                    pt[:, :gw], pt[:, :gw],
                    msk_s[:, MOFF[grp[0]]:MOFF[grp[0]] + gw])
                pts[i] = pt

            def emit_o(i):
                h, grp = all_groups[i]
                nd = _needed(grp[0])
                pt = pts.pop(i)
                for j, t in enumerate(grp):
                    for p in reversed(range(nd)):
                        nc.tensor.matmul(
                            o_by_head[h][p][:],
                            lhsT=pt[:, j * 128 * nd + p * 128:
                                    j * 128 * nd + (p + 1) * 128],
                            rhs=vsb[:, t * VROW:(t + 1) * VROW],
                            start=(t == 0),
                            stop=(t == NCOV[p] - 1),
                        )

            evq = []   # (h, p, en) awaiting their PE transposes

            def emit_evict_scale(i):
                h, grp = all_groups[i]
                for p in range(NSLOT):
                    if NCOV[p] - 1 != grp[-1]:
                        continue
                    o_tile = o_by_head[h][p]
                    r = rcpp.tile([128, 1], fp32, tag="r")
                    nc.vector.reciprocal(r[:], o_tile[:, HEAD_DIM:VROW])
                    en = enp.tile([128, HEAD_DIM], bf16, tag="en")
                    nc.scalar.activation(en[:], o_tile[:, 0:HEAD_DIM], Copy,
                                         scale=r[:])
                    evq.append((h, p, en))

            def emit_transposes():
                # XBAR DMA transposes: keeps the tensor engine and DVE out
                # of the eviction path entirely (SP + DMA are idle here).
                while evq:
                    h, p, en = evq.pop(0)
                    for hh in range(2):
                        nc.sync.dma_start_transpose(
                            out=enct[:, (2 * h + hh) * QBLK + p * 128:
                                     (2 * h + hh) * QBLK + (p + 1) * 128],
                            in_=en[:, hh * 128:(hh + 1) * 128])

            # wf prefetch: triggers fire on the idle SP queue during attention
            load_w(wf, 0)
            load_w(wf, 1)

            NG = len(all_groups)
            LAG = 2
            for i in range(NG):
                emit_s(i)
                if i >= LAG:
                    emit_o(i - LAG)
                    emit_transposes()       # drain earlier evictions
                    emit_evict_scale(i - LAG)

            # Attention tail interleaved with the first final-proj stripes:
            # heads 0..8 (k=0..17) of stripes 0/1 accumulate while head 9's
            # last O/eviction chains drain, hiding their latency.
            fin_ps = {}

            def final_partial(m, kr):
                if m not in fin_ps:
                    fin_ps[m] = stp.tile([128, QBLK], fp32, tag="st",
                                         name=f"fps{m}")
                for k in kr:
                    nc.tensor.matmul(
                        fin_ps[m][:],
                        lhsT=wq_tiles[("wf", m)][:, k * 128:(k + 1) * 128],
                        rhs=enct[:, k * QBLK:(k + 1) * QBLK],
                        start=(k == 0),
                        stop=(k == NW - 1),
                    )

            emit_o(NG - 2)
            emit_evict_scale(NG - 2)
            final_partial(0, range(0, 18))
            emit_o(NG - 1)
            emit_evict_scale(NG - 1)
            final_partial(1, range(0, 18))
            emit_transposes()

            # ---- final projection: out^T = wf @ enc^T + bias ----
            for m in range(NW):
                if m + 2 < NW:
                    load_w(wf, m + 2)
                wf_m = wq_tiles.pop(("wf", m))
                if m in fin_ps:  # heads 0..8 already accumulated above
                    ps = fin_ps.pop(m)
                    for k in range(18, NW):
                        nc.tensor.matmul(
                            ps[:],
                            lhsT=wf_m[:, k * 128:(k + 1) * 128],
                            rhs=enct[:, k * QBLK:(k + 1) * QBLK],
                            start=(k == 0),
                            stop=(k == NW - 1),
                        )
                    osb = outp.tile([128, QBLK], fp32, tag="osb")
                    nc.vector.tensor_scalar_add(osb[:], ps[:], bia_s[:, m:m + 1])
                    nc.sync.dma_start(out=out[m], in_=osb[:])
                    continue
                ps = pp.tile([128, QBLK], fp32, tag="pp")
                if m < NW - 1:
                    for k in range(NW):
                        nc.tensor.matmul(
                            ps[:],
                            lhsT=wf_m[:, k * 128:(k + 1) * 128],
                            rhs=enct[:, k * QBLK:(k + 1) * QBLK],
                            start=(k == 0),
                            stop=(k == NW - 1),
                        )
                    osb = outp.tile([128, QBLK], fp32, tag="osb")
                    nc.vector.tensor_scalar_add(osb[:], ps[:], bia_s[:, m:m + 1])
                    nc.sync.dma_start(out=out[m], in_=osb[:])
                else:
                    # last stripe in column halves (separate PSUM tiles): the
                    # first half's bias/store drains under the second half
                    osb = outp.tile([128, QBLK], fp32, tag="osb")
                    for j2 in range(2):
                        cj = slice(256 * j2, 256 * (j2 + 1))
                        psj = ps if j2 == 0 else pp.tile(
                            [128, QBLK], fp32, tag="pp")
                        for k in range(NW):
                            nc.tensor.matmul(
                                psj[:, 0:256],
                                lhsT=wf_m[:, k * 128:(k + 1) * 128],
                                rhs=enct[:, k * QBLK + 256 * j2:
                                         k * QBLK + 256 * (j2 + 1)],
                                start=(k == 0),
                                stop=(k == NW - 1),
                            )
                        nc.vector.tensor_scalar_add(
                            osb[:, cj], psj[:, 0:256], bia_s[:, m:m + 1])
                        nc.sync.dma_start(out=out[m][:, cj], in_=osb[:, cj])

    if not nc.is_finalized():
        nc.finalize()  # bacc register allocation — required before walrus compile
    return nc


def get_nc():
    if "nc" not in _NC_CACHE:
        _NC_CACHE["nc"] = _build_nc()
    return _NC_CACHE["nc"]


def _chunk_of_slot(r, p):
    """Physical 128-token query chunk held by rank r's slot p."""
    return r + 12 - 4 * p


def _host_prepare(x, segment_pos, wq, wk, wv, w_final, b_final):
    """Build shared + per-core device input arrays."""
    x = np.asarray(x, dtype=np.float32)
    segment_pos = np.asarray(segment_pos)
    wq = np.asarray(wq, dtype=np.float32)
    wk = np.asarray(wk, dtype=np.float32)
    wv = np.asarray(wv, dtype=np.float32)
    w_final = np.asarray(w_final, dtype=np.float32)
    b_final = np.asarray(b_final, dtype=np.float32)

    def stripes_sq(w):  # [WIDTH, WIDTH] torch-Linear weight -> [20,128,WIDTH] w^T stripes
        wt = np.ascontiguousarray(w.T)
        return np.ascontiguousarray(
            wt.reshape(NW, 128, NW, 128).transpose(2, 1, 0, 3).reshape(NW, 128, WIDTH)
        ).astype(BF16)

    def packed_kv(w):  # [HEAD_DIM, WIDTH] -> [128, NW*HEAD_DIM] w^T stripes
        return np.ascontiguousarray(
            w.T.reshape(NW, 128, HEAD_DIM).transpose(1, 0, 2).reshape(
                128, NW * HEAD_DIM)
        ).astype(BF16)

    shared = {
        "wq": stripes_sq(wq),
        "wf": stripes_sq(w_final),
        "wkr": packed_kv(wk),
        "wvr": packed_kv(wv),
        "bia": np.ascontiguousarray(b_final.reshape(NW, 128).T).astype(np.float32),
    }

    inv_freq = (
        1.0 / MAX_WAVELENGTH ** (2.0 * np.arange(HEAD_DIM // 4, dtype=np.float32)
                                 / (HEAD_DIM // 2))
    ).astype(np.float32)

    def cossin(pos):
        ang = inv_freq[:, None] * pos[None, :].astype(np.float32)
        return (np.cos(ang).astype(np.float32), np.sin(ang).astype(np.float32))

    in_maps = []
    for c in range(8):
        b, r = c // 4, c % 4
        pos = segment_pos[b].astype(np.float32)
        seg = np.cumsum((segment_pos[b] == 0).astype(np.int64))

        qidx = np.concatenate(
            [np.arange(128) + 128 * _chunk_of_slot(r, p) for p in range(NSLOT)])
        kidx = np.arange(KVB) + KVB * r

        xqc = np.ascontiguousarray(x[b][qidx].T).astype(BF16).reshape(NW, 128, QBLK)
        xkvc = np.ascontiguousarray(x[b][kidx].T).astype(BF16).reshape(NW, 128, KVB)

        csq_, snq_ = cossin(pos[qidx])
        csk_, snk_ = cossin(pos[kidx])

        # packed masks: for key tile t, needed(t) slot blocks of [128k,128q]
        mask = np.zeros((128, MCOLS), dtype=BF16)
        for t in range(NTT):
            tk = np.arange(128) + 128 * t
            for p in range(_needed(t)):
                tq = qidx[p * 128:(p + 1) * 128]
                allow = (
                    (tk[:, None] <= tq[None, :])
                    & (tq[None, :] <= tk[:, None] + WINDOW)
                    & (seg[tk][:, None] == seg[tq][None, :])
                )
                mask[:, MOFF[t] + 128 * p: MOFF[t] + 128 * (p + 1)] = allow
        in_maps.append(dict(
            shared,
            xq=xqc,
            xkv=xkvc,
            csk=csk_,
            snk=snk_,
            csq=csq_,
            snq=snq_,
            msk=mask,
        ))
    return in_maps


def _assemble(results):
    out = np.empty((B, T, WIDTH), dtype=np.float32)
    for c, res in enumerate(results):
        b, r = c // 4, c % 4
        o = np.asarray(res["out"], dtype=np.float32)  # [NW, 128, QBLK]
        for p in range(NSLOT):
            ch = _chunk_of_slot(r, p)
            out[b, 128 * ch:128 * (ch + 1), :] = (
                o[:, :, 128 * p:128 * (p + 1)].transpose(2, 0, 1).reshape(128, WIDTH)
            )
    return out


def kernel(x, segment_pos, wq, wk, wv, w_final, b_final):
    from concourse.bass_utils import run_bass_kernel_spmd

    nc = get_nc()
    in_maps = _host_prepare(x, segment_pos, wq, wk, wv, w_final, b_final)
    res = run_bass_kernel_spmd(nc, in_maps, list(range(8)))
    return _assemble(res.results)
